# revision 50
# baseline (speedup 1.0000x reference)
"""Trainium2 Bass kernel for BaseBidirectionalAttention (BiDAF-style attention).

Reference computation (per batch b):
    sim[c,q]  = <w_c, ctx_c> + <w_q, q_q> + <w_m, ctx_c * q_q>
    c2q       = softmax_q(sim) @ question                      (C, E)
    q2c_w     = softmax_c(max_q sim)                           (C,)
    q2c       = q2c_w @ context                                (E,)
    attended  = [ctx, c2q, ctx*c2q, ctx*q2c]                   (C, 4E)
    out       = (attended @ final_W.T + final_b) * mask[:,None] (C, 4E)

Sharding: data-parallel over batch. 32 batches / 8 cores = 4 per core.
All parameters (final_W etc., <5MB) replicated on every core.

Device-side layouts (prepared host-side; pure layout transforms):
    ctxT16 : context^T   (B, E, C)  bf16 (att3 source, final lhsT)
    ctxTh  : context^T   (B, E, C)  fp16 (sim lhsT: fp16 keeps softmax
             logits ~8x more exact than bf16 at the same PE rate)
    ctxn16 : context     (B, C, E)  bf16 (q2c contraction)
    qT16   : question^T  (B, E, Q)  bf16 (qw2 lhsT)
    qTh    : question^T  (B, E, Q)  fp16 (sim rhs source)
    q16    : question    bf16       (c2q lhsT)
    wt16   : final_W^T   (4E, 4E)   bf16 (final-matmul moving operand)

Key structure:
  - attended^T is built on-chip in bf16; the final matmul needs only
    K=576 of 1024: block4 (ctx*q2c) is folded into block1's weights
    (wtm = W1^T + q2c . W4^T) and block2 (c2q@W2^T = P @ (question@W2^T))
    contracts over Q=64 via qw2.
  - The final matmul runs lhsT-major over both 512-wide psum halves so
    each stationary operand is loaded once per two matmuls (the PE's
    double-buffered weights absorb the reload).
  - All matmuls are 16-bit (measured ~0.55-0.62 ns/psum-row on this HW vs
    2.5 ns/row fp32): sim logits in fp16 (bf16 logits flip near-tied
    softmax argmaxes and blow the max-norm error), the rest bf16.
  - q2c is contracted with ctx as 16 N=1 column matmuls (lhsT = ctx
    chunks) instead of 8 N=256 row matmuls -- PE cost is moving-row
    driven, so the column orientation is ~4x cheaper and lands q2c as
    the per-partition scalar wtm needs.
  - P^T via PE transposes, two 64-col chunks per 128x128 transpose.
  - Engine balance: PE does matmuls only; ACT does ONLY exps and half the
    final-matmul PSUM evictions; DVE does reduces + the other eviction
    half; Pool (gpsimd) does all SBUF-only elementwise work.
  - Softmax: one exp per 4-row-chunk half with a per-partition *block*
    max as bias (renormalization cancels exactly), p in bf16.
  - Schedule per batch: simMMs(b) | final(b-1) all tiles | softmax-
    dependent PE ops(b) -- PE never waits on the softmax chain.
  - Output is written bf16 (halves DMA); the host upcasts to fp32.
"""

import numpy as np
import ml_dtypes

import concourse.bass as bass
import concourse.mybir as mybir
import concourse.tile as tile
from concourse.bass_utils import run_bass_kernel_spmd
from concourse.masks import make_identity

B, C, Q, E = 32, 1024, 64, 256
FE = 4 * E
NCORES = 8
BL = B // NCORES  # batches per core

F32 = mybir.dt.float32
BF16 = mybir.dt.bfloat16
F16 = mybir.dt.float16
AX = mybir.AxisListType.X
AXY = mybir.AxisListType.XY
ALU = mybir.AluOpType
ACTF = mybir.ActivationFunctionType


def _split_multi_waits(nc):
    """The walrus build in this environment supports a single sync-wait per
    instruction. Move extra waits onto preceding same-engine NoOps."""
    counter = 0
    for f in nc.m.functions:
        for bb in f.blocks:
            insts = bb.instructions
            i = 0
            while i < len(insts):
                inst = insts[i]
                si = inst.sync_info
                waits = list(si.on_wait) if si is not None and si.on_wait else []
                if len(waits) > 1:
                    inst.sync_info = mybir.SyncInfo(
                        on_wait=[waits[-1]],
                        on_update=list(si.on_update) if si.on_update else [],
                    )
                    for w in waits[:-1]:
                        nop = mybir.InstNoOp(
                            name=f"I-swsplit-{counter}", engine=inst.engine
                        )
                        counter += 1
                        nop.sync_info = mybir.SyncInfo(on_wait=[w], on_update=[])
                        nc.register_instruction(nop)
                        insts.insert(i, nop)
                        i += 1
                i += 1


def _emit(nc, tc, dram, ctx, loop=1, bias_zero=False, mask_ones=False, probe=None,
          unroll=None):
    consts = ctx.enter_context(tc.tile_pool(name="consts", bufs=1))
    inp = ctx.enter_context(tc.tile_pool(name="inp", bufs=4))
    work = ctx.enter_context(tc.tile_pool(name="work", bufs=3))
    small = ctx.enter_context(tc.tile_pool(name="small", bufs=6))
    outp = ctx.enter_context(tc.tile_pool(name="outp", bufs=3))
    # PSUM: 8 banks total. Dedicated pools per phase so batch N+1's
    # attention never waits on batch N's final-matmul psums.
    ps_sim = ctx.enter_context(tc.tile_pool(name="ps_sim", bufs=1, space="PSUM"))
    ps_tp = ctx.enter_context(tc.tile_pool(name="ps_tp", bufs=1, space="PSUM"))
    ps_fin = ctx.enter_context(tc.tile_pool(name="ps_fin", bufs=4, space="PSUM"))
    ps_msc = ctx.enter_context(tc.tile_pool(name="ps_msc", bufs=2, space="PSUM"))

    # ---- constants ----
    wt = consts.tile([128, 8, FE], BF16)  # final_W^T, k-chunk major
    nc.sync.dma_start(wt[:], dram["wt16"].rearrange("(k p) f -> p k f", p=128))
    vecs = consts.tile([128, 2, 4], F32)  # cols: wq, wc, wm, 0 (e-chunked)
    nc.sync.dma_start(vecs[:], dram["vecs"].rearrange("(c p) v -> p c v", p=128))
    vecs16 = consts.tile([128, 2, 4], F16)
    nc.gpsimd.tensor_copy(vecs16[:], vecs[:])
    ones_f = consts.tile([1, 128], F32)
    nc.vector.memset(ones_f[:], 1.0)
    ones16 = consts.tile([1, 128], F16)
    nc.vector.memset(ones16[:], 1.0)
    ones_col = consts.tile([128, 1], F32)
    nc.vector.memset(ones_col[:], 1.0)
    ident = consts.tile([128, 128], F32)
    make_identity(nc, ident[:])
    ident16 = consts.tile([128, 128], BF16)
    nc.gpsimd.tensor_copy(ident16[:], ident[:])
    if not bias_zero:
        b_b128 = consts.tile([128, FE], BF16)
        nc.sync.dma_start(
            b_b128[:],
            bass.AP(
                tensor=dram["b16"].tensor,
                offset=dram["b16"].offset,
                ap=[[0, 128]] + list(dram["b16"].ap),
            ),
        )

    if probe == "dma":
        dma_src = [
            consts.tile([128, FE], BF16, tag=f"dmasrc{i}", name=f"dmasrc{i}")
            for i in range(2)
        ]
        for t in dma_src:
            nc.gpsimd.memset(t[:], 1.0)

    if probe == "pestream":
        pfake = consts.tile([128, 8, Q], BF16, tag="pfake")
        nc.gpsimd.memset(pfake[:], 0.01)
        e16fake = consts.tile([128, 8], BF16, tag="e16fake")
        nc.gpsimd.memset(e16fake[:], 0.01)

    if probe in ("peclock", "peclock2"):
        mmw = [
            consts.tile([128, 128], BF16, tag=f"mmw{i}", name=f"mmw{i}")
            for i in range(2)
        ]
        for t in mmw:
            nc.gpsimd.memset(t[:], 0.01)

    def _attn_A(b, pair_st=None):
        """Inputs + similarity matmuls + row softmax + qw2 (no PE ops that
        wait on the softmax chain)."""
        ctxT16 = inp.tile([128, 2, C], BF16, tag="ctxT16")
        nc.sync.dma_start(
            ctxT16[:], dram["ctxT16"][b].rearrange("(c p) n -> p c n", p=128)
        )
        ctxTh = inp.tile([128, 2, C], F16, tag="ctxTh")
        nc.sync.dma_start(
            ctxTh[:], dram["ctxTh"][b].rearrange("(c p) n -> p c n", p=128)
        )
        qTh = inp.tile([128, 2, Q], F16, tag="qTh")
        nc.sync.dma_start(
            qTh[:], dram["qTh"][b].rearrange("(c p) q -> p c q", p=128)
        )
        qT16 = inp.tile([128, 2, Q], BF16, tag="qT16")
        nc.sync.dma_start(
            qT16[:], dram["qT16"][b].rearrange("(c p) q -> p c q", p=128)
        )
        q16 = inp.tile([64, E], BF16, tag="q16")
        nc.sync.dma_start(q16[:], dram["q16"][b])
        ctxn16 = inp.tile([128, 8, E], BF16, tag="ctxn16")
        nc.sync.dma_start(
            ctxn16[:], dram["ctxn16"][b].rearrange("(j p) e -> p j e", p=128)
        )
        if not mask_ones:
            mask_row = inp.tile([1, C], F32, tag="mask")
            nc.sync.dma_start(mask_row[:], dram["mask"][b : b + 1, :])

        if probe == "dma":
            for cs in range(8):
                nc.sync.dma_start(
                    dram["out"][b, cs * 128 : (cs + 1) * 128, :], dma_src[cs % 2][:]
                )
            return None

        if probe in ("peclock", "peclock2"):
            # Pure PE streaming-rate probe: 96 N=512 bf16 matmuls per batch
            # over 4 psum banks (24-deep accumulation, one eviction per bank).
            # peclock: one stationary operand forever -> no reload cost.
            # peclock2: alternate two stationary operands -> reload per mm.
            for bank in range(4):
                ps_o = ps_fin.tile([128, 512], F32, tag="fin")
                for i in range(24):
                    lhsT = mmw[i % 2 if probe == "peclock2" else 0]
                    nc.tensor.matmul(
                        ps_o[:], lhsT[:], wt[:, i % 8, 0:512],
                        start=(i == 0), stop=(i == 23),
                    )
                s = small.tile([128, 1], F32, tag="mmsink")
                nc.vector.tensor_copy(s[:], ps_o[:, 0:1])
            return None

        if probe in ("mmonly", "mm256"):
            # dense self-loading bf16 matmuls, minimal eviction: HW ns/MM
            nf = 2 if probe == "mmonly" else 4
            fw = FE // nf
            for cs in range(8):
                csl = slice(cs * 128, (cs + 1) * 128)
                for fh in range(nf):
                    fhl = slice(fh * fw, (fh + 1) * fw)
                    ps_o = ps_fin.tile([128, fw], F32, tag="fin")
                    for kc in range(8):
                        nc.tensor.matmul(
                            ps_o[:], ctxT16[:, kc % 2, csl], wt[:, kc, fhl],
                            start=(kc == 0), stop=(kc == 7),
                        )
                    s = small.tile([128, 1], F32, tag="mmsink")
                    nc.vector.tensor_copy(s[:], ps_o[:, 0:1])
            return None

        if probe == "final":
            for cs in range(8):
                csl = slice(cs * 128, (cs + 1) * 128)
                out_s = outp.tile([128, FE], BF16, tag="out")
                for fh in range(2):
                    fhl = slice(fh * 512, (fh + 1) * 512)
                    ps_o = ps_fin.tile([128, 512], F32, tag="fin")
                    for kc in range(8):
                        nc.tensor.matmul(
                            ps_o[:], ctxT16[:, kc % 2, csl], wt[:, kc, fhl],
                            start=(kc == 0), stop=(kc == 7),
                        )
                    if fh == 0:
                        nc.scalar.copy(out_s[:, fhl], ps_o[:])
                    else:
                        nc.vector.tensor_copy(out_s[:, fhl], ps_o[:])
                nc.sync.dma_start(dram["out"][b, csl, :], out_s[:])
            return None

        if probe == "pestream":
            # The full per-batch PE stream with softmax/chain deps severed:
            # isolates how fast the PE mix itself can stream on HW.
            rhs_ext = work.tile([128, 2, Q + 1], F16, tag="qTs")
            for ec in range(2):
                nc.gpsimd.tensor_scalar_mul(
                    rhs_ext[:, ec, 0:Q], qTh[:, ec, :], vecs[:, ec, 2:3]
                )
                nc.gpsimd.tensor_copy(rhs_ext[:, ec, Q : Q + 1], vecs16[:, ec, 1:2])
            ps_qw = ps_msc.tile([1, Q], F32, tag="msc")
            for ec in range(2):
                nc.tensor.matmul(
                    ps_qw[:], vecs16[:, ec, 0:1], qTh[:, ec, :],
                    start=(ec == 0), stop=(ec == 1),
                )
            qsk = small.tile([1, 1], F32, tag="qwsink")
            nc.vector.tensor_copy(qsk[:], ps_qw[:, 0:1])
            for h in range(2):
                ps_s = ps_sim.tile([128, 4, Q + 1], F32, tag="sim")
                for k in range(4):
                    cs = h * 4 + k
                    csl = slice(cs * 128, (cs + 1) * 128)
                    nc.tensor.matmul(
                        ps_s[:, k, :], ctxTh[:, 0, csl], rhs_ext[:, 0, :],
                        start=(k == 0), stop=False,
                    )
                    nc.tensor.matmul(
                        ps_s[:, k, :], ctxTh[:, 1, csl], rhs_ext[:, 1, :],
                        start=False, stop=False,
                    )
                    nc.tensor.matmul(
                        ps_s[:, k, :], ones16[:], rhs_ext[0:1, 0, :],
                        start=False, stop=(k == 3),
                    )
                ssk = small.tile([128, 1], F32, tag="ssink")
                nc.vector.tensor_copy(ssk[:], ps_s[:, 0, 0:1])
            qw2 = work.tile([64, FE], BF16, tag="qw2")
            for fh in range(2):
                fhl = slice(fh * 512, (fh + 1) * 512)
                ps_q = ps_msc.tile([64, 512], F32, tag="msc")
                for ec in range(2):
                    nc.tensor.matmul(
                        ps_q[:], qT16[:, ec, :], wt[:, ec + 2, fhl],
                        start=(ec == 0), stop=(ec == 1),
                    )
                nc.scalar.copy(qw2[:, fhl], ps_q[:])
            pnt = work.tile([64, C], BF16, tag="pnt")
            for h in range(2):
                ps_t = ps_tp.tile([64, 4, 128], BF16, tag="tp")
                for k in range(4):
                    nc.tensor.transpose(
                        ps_t[:, k, :], pfake[:, h * 4 + k, :], ident16[:]
                    )
                nc.vector.tensor_copy(
                    pnt[:, h * 512 : (h + 1) * 512],
                    ps_t[:].rearrange("p a b -> p (a b)"),
                )
            att3 = work.tile([128, 2, C], BF16, tag="att3")
            for ec in range(2):
                for ch in range(2):
                    chl = slice(ch * 512, (ch + 1) * 512)
                    ps_c2q = ps_msc.tile([128, 512], F32, tag="msc")
                    nc.tensor.matmul(
                        ps_c2q[:], q16[:, ec * 128 : (ec + 1) * 128], pnt[:, chl],
                        start=True, stop=True,
                    )
                    nc.vector.tensor_mul(
                        att3[:, ec, chl], ctxT16[:, ec, chl], ps_c2q[:]
                    )
            for ec in range(2):
                ps_qc = ps_msc.tile([128, 1], F32, tag="msc")
                for j in range(8):
                    nc.tensor.matmul(
                        ps_qc[:],
                        ctxn16[:, j, ec * 128 : (ec + 1) * 128],
                        e16fake[:, j : j + 1],
                        start=(j == 0), stop=(j == 7),
                    )
                q2sk = small.tile([1, 1], F32, tag="q2sink")
                nc.vector.tensor_copy(q2sk[:], ps_qc[0:1, 0:1])
            for cs in range(8):
                csl = slice(cs * 128, (cs + 1) * 128)
                out_s = outp.tile([128, FE], BF16, tag="out")
                for fh in range(2):
                    fhl = slice(fh * 512, (fh + 1) * 512)
                    ps_o = ps_fin.tile([128, 512], F32, tag="fin")
                    nc.tensor.matmul(
                        ps_o[:], pnt[:, csl], qw2[:, fhl], start=True, stop=False
                    )
                    nc.tensor.matmul(
                        ps_o[:], att3[:, 0, csl], wt[:, 4, fhl],
                        start=False, stop=False,
                    )
                    nc.tensor.matmul(
                        ps_o[:], att3[:, 1, csl], wt[:, 5, fhl],
                        start=False, stop=False,
                    )
                    nc.tensor.matmul(
                        ps_o[:], ctxT16[:, 0, csl], wt[:, 0, fhl],
                        start=False, stop=False,
                    )
                    nc.tensor.matmul(
                        ps_o[:], ctxT16[:, 1, csl], wt[:, 1, fhl],
                        start=False, stop=True,
                    )
                    if fh == 0:
                        nc.scalar.copy(out_s[:, fhl], ps_o[:])
                    else:
                        nc.vector.tensor_copy(out_s[:, fhl], ps_o[:])
                nc.sync.dma_start(dram["out"][b, csl, :], out_s[:])
            return None

        # ---- rhs_ext = [qT * w_multiple | w_context]; col Q -> ctxw ----
        # (built on Pool: keeps DVE's in-order queue clear for the psum
        # evictions of the previous batch's final matmul)
        rhs_ext = work.tile([128, 2, Q + 1], F16, tag="qTs")
        for ec in range(2):
            nc.gpsimd.tensor_scalar_mul(
                rhs_ext[:, ec, 0:Q], qTh[:, ec, :], vecs[:, ec, 2:3]
            )
            nc.gpsimd.tensor_copy(rhs_ext[:, ec, Q : Q + 1], vecs16[:, ec, 1:2])

        # ---- q_weighted row: qw[q] = <w_question, question_q> ----
        ps_qw = ps_msc.tile([1, Q], F32, tag="msc")
        for ec in range(2):
            nc.tensor.matmul(
                ps_qw[:], vecs16[:, ec, 0:1], qTh[:, ec, :],
                start=(ec == 0), stop=(ec == 1),
            )
        # qw repeated 4x so ONE broadcast matmul covers a whole psum half
        qw_ext4 = work.tile([1, 4, Q + 1], F16, tag="qw")  # [qw | 0] x 4
        nc.vector.memset(qw_ext4[:], 0.0)
        for j in range(4):
            nc.vector.tensor_copy(qw_ext4[0:1, j, 0:Q], ps_qw[:])

        # ---- mask columns ----
        mask_c = None
        if not mask_ones:
            mask_c = work.tile([128, 8], F32, tag="mask_c")
            for cs in range(8):
                ps_mc = ps_msc.tile([128, 1], F32, tag="msc")
                nc.tensor.matmul(
                    ps_mc[:],
                    mask_row[0:1, cs * 128 : (cs + 1) * 128],
                    ones_f[0:1, 0:1],
                    start=True,
                    stop=True,
                )
                nc.vector.tensor_copy(mask_c[:, cs : cs + 1], ps_mc[:])

        # ---- similarity: logits (mult + qw[q]) cols 0..Q, ctxw[c] in col Q.
        # One exp per half with a per-partition BLOCK max as bias -- the
        # 1/Z normalization cancels the (rowmax - blockmax) offset exactly.
        negrow = work.tile([128, 8], F32, tag="negrow")
        bias_h = work.tile([128, 2], F32, tag="bias_h")
        ctxw_c = work.tile([128, 8], F32, tag="ctxw_c")
        p = work.tile([128, 8, Q], BF16, tag="p")
        zrows = work.tile([128, 8], F32, tag="zrows")
        rz = work.tile([128, 8], F32, tag="rz")
        for h in range(2):
            ps_s = ps_sim.tile([128, 4, Q + 1], F32, tag="sim")
            for k in range(4):
                cs = h * 4 + k
                csl = slice(cs * 128, (cs + 1) * 128)
                nc.tensor.matmul(
                    ps_s[:, k, :], ctxTh[:, 0, csl], rhs_ext[:, 0, :],
                    start=(k == 0), stop=False,
                )
                nc.tensor.matmul(
                    ps_s[:, k, :], ctxTh[:, 1, csl], rhs_ext[:, 1, :],
                    start=False, stop=False,
                )
            nc.tensor.matmul(
                ps_s[:, :, :], ones16[:], qw_ext4[0:1, :, :],
                start=False, stop=True,
            )
            hl = slice(h * 4, (h + 1) * 4)
            nc.vector.reduce_max(
                out=negrow[:, hl], in_=ps_s[:, :, 0:Q], axis=AX, negate=True
            )
            nc.vector.reduce_max(
                out=bias_h[:, h : h + 1], in_=ps_s[:, :, 0:Q], axis=AXY, negate=True
            )
            nc.vector.tensor_copy(
                ctxw_c[:, hl], ps_s[:, :, Q : Q + 1].rearrange("p a b -> p (a b)")
            )
            nc.scalar.activation(
                out=p[:, hl, :],
                in_=ps_s[:, :, 0:Q],
                func=ACTF.Exp,
                bias=bias_h[:, h : h + 1],
                scale=1.0,
            )
            nc.vector.reduce_sum(out=zrows[:, hl], in_=p[:, hl, :], axis=AX)
        # ---- q2c chain part A (serial small-op chain: kick it off as early
        # as possible so wtm construction in attn_B never gates the final
        # matmul). Produces e16 = exp(rowmax - gmax) and zrow2. ----
        rowtrue = work.tile([128, 8], F32, tag="rowtrue")
        nc.vector.tensor_sub(rowtrue[:], ctxw_c[:], negrow[:])
        # gmax in bf16 is fine: it is applied as the SAME bias to every row,
        # so any common inexactness cancels in the q2c softmax normalization.
        colmax = small.tile([128, 1], BF16, tag="colmax")
        nc.vector.reduce_max(out=colmax[:], in_=rowtrue[:], axis=AX)
        ps_t1 = ps_msc.tile([1, 128], BF16, tag="msc")
        nc.tensor.transpose(ps_t1[:], colmax[:], ident16[:])
        tmax = small.tile([1, 128], F32, tag="tmax")
        nc.vector.tensor_copy(tmax[:], ps_t1[:])
        gneg = small.tile([1, 1], F32, tag="gneg")  # -gmax
        nc.vector.reduce_max(out=gneg[:], in_=tmax[:], axis=AX, negate=True)
        ps_gb = ps_msc.tile([128, 1], F32, tag="msc")
        nc.tensor.matmul(ps_gb[:], ones_f[:], gneg[:], start=True, stop=True)
        gneg_col = small.tile([128, 1], F32, tag="gnegc")
        nc.vector.tensor_copy(gneg_col[:], ps_gb[:])
        e_t = work.tile([128, 8], F32, tag="e_t")  # exp(rowtrue - gmax)
        zrow2 = small.tile([128, 1], F32, tag="zrow2")
        nc.scalar.activation(
            out=e_t[:],
            in_=rowtrue[:],
            func=ACTF.Exp,
            bias=gneg_col[:],
            scale=1.0,
            accum_out=zrow2[:],
        )
        e16 = work.tile([128, 8], BF16, tag="e16")
        nc.gpsimd.tensor_copy(e16[:], e_t[:])

        nc.vector.reciprocal(rz[:], zrows[:])
        for cs in range(8):
            nc.gpsimd.tensor_scalar_mul(p[:, cs, :], p[:, cs, :], rz[:, cs : cs + 1])

        # ---- qW2 = question @ W2^T (+ bias: softmax rows sum to 1, so
        # adding b here adds exactly b to the output) ----
        qw2 = work.tile([64, FE], BF16, tag="qw2")
        for fh in range(2):
            fhl = slice(fh * 512, (fh + 1) * 512)
            ps_q = ps_msc.tile([64, 512], F32, tag="msc")
            for ec in range(2):
                nc.tensor.matmul(
                    ps_q[:], qT16[:, ec, :], wt[:, ec + 2, fhl],
                    start=(ec == 0), stop=(ec == 1),
                )
            if bias_zero:
                nc.scalar.copy(qw2[:, fhl], ps_q[:])
            else:
                nc.vector.tensor_add(qw2[:, fhl], ps_q[:], b_b128[0:64, fhl])

        return dict(
            b=b, ctxT16=ctxT16, q16=q16, ctxn16=ctxn16, mask_c=mask_c,
            p=p, qw2=qw2, e16=e16, zrow2=zrow2,
        )

    def _attn_B1(st):
        """P^T via PE transposes. Chunk PAIRS go through one [128,128]
        transpose (half the PE moving rows); the pair lands as q-rows 0:64
        (even chunk) / 64:128 (odd chunk) and is split at eviction."""
        p = st["p"]
        pnt = work.tile([64, C], BF16, tag="pnt")  # P_norm^T
        for h in range(2):
            ps_t = ps_tp.tile([128, 2, 128], BF16, tag="tp")
            for k in range(2):
                cs = h * 4 + 2 * k
                nc.tensor.transpose(
                    ps_t[:, k, :],
                    p[:, cs : cs + 2, :].rearrange("p a b -> p (a b)"),
                    ident16[:],
                )
            for k in range(2):
                cs = h * 4 + 2 * k
                nc.scalar.copy(
                    pnt[:, cs * 128 : (cs + 1) * 128], ps_t[0:64, k, :]
                )
                nc.scalar.copy(
                    pnt[:, (cs + 1) * 128 : (cs + 2) * 128], ps_t[64:128, k, :]
                )
        st["pnt"] = pnt

    def _attn_B2(st):
        """Remaining softmax-dependent ops: c2q, q2c, merged weights."""
        ctxT16, q16, ctxn16 = st["ctxT16"], st["q16"], st["ctxn16"]
        e16, zrow2, pnt = st["e16"], st["zrow2"], st["pnt"]

        # ---- c2q attention (only needed for block3 = ctx * c2q) ----
        att3 = work.tile([128, 2, C], BF16, tag="att3")  # (ctx*c2q)^T
        for ec in range(2):
            for ch in range(2):
                chl = slice(ch * 512, (ch + 1) * 512)
                ps_c2q = ps_msc.tile([128, 512], F32, tag="msc")
                nc.tensor.matmul(
                    ps_c2q[:],
                    q16[:, ec * 128 : (ec + 1) * 128],
                    pnt[:, chl],
                    start=True,
                    stop=True,
                )
                nc.vector.tensor_mul(att3[:, ec, chl], ctxT16[:, ec, chl], ps_c2q[:])

        # ---- q2c chain part B (e16/zrow2 were produced back in attn_A) ----
        ps_z = ps_msc.tile([1, 1], F32, tag="msc")
        nc.tensor.matmul(ps_z[:], zrow2[:], ones_col[:], start=True, stop=True)
        z_s = small.tile([1, 1], F32, tag="z_s")
        nc.vector.tensor_copy(z_s[:], ps_z[:])
        rz1 = small.tile([1, 1], F32, tag="rz1")
        nc.vector.reciprocal(rz1[:], z_s[:])
        # q2c computed TRANSPOSED: out [e, 1] columns directly (N=1 matmuls
        # are ~free: cost ~ per-instruction overhead, not 256 moving rows),
        # which also skips the row->column transpose matmuls for wtm.
        # block4 never materializes: (ctx . q2c) @ W4^T == ctx @ (q2c . W4^T),
        # so fold q2c into merged weights for the ctx block instead.
        wtm = work.tile([128, 2, FE], BF16, tag="wtm")  # W1^T + q2c . W4^T
        ps_rz = ps_msc.tile([128, 1], F32, tag="msc")
        nc.tensor.matmul(ps_rz[:], ones_f[:], rz1[:], start=True, stop=True)
        rz_col = small.tile([128, 1], F32, tag="rz_col")
        nc.vector.tensor_copy(rz_col[:], ps_rz[:])
        q2c_col2 = small.tile([128, 2], F32, tag="q2c_col2")
        for ec in range(2):
            ps_qc = ps_msc.tile([128, 1], F32, tag="msc")
            for j in range(8):
                nc.tensor.matmul(
                    ps_qc[:],
                    ctxn16[:, j, ec * 128 : (ec + 1) * 128],
                    e16[:, j : j + 1],
                    start=(j == 0),
                    stop=(j == 7),
                )
            # 1/Z folded into the eviction copy
            nc.vector.tensor_scalar_mul(q2c_col2[:, ec : ec + 1], ps_qc[:], rz_col[:])
        for ec in range(2):
            nc.gpsimd.tensor_scalar_mul(
                wtm[:, ec, :], wt[:, ec + 6, :], q2c_col2[:, ec : ec + 1]
            )
            nc.gpsimd.tensor_add(wtm[:, ec, :], wtm[:, ec, :], wt[:, ec, :])

        st["att3"], st["wtm"] = att3, wtm

        if probe == "attn":
            b = st["b"]
            sink = outp.tile([128, 32], BF16, tag="sink")
            nc.vector.tensor_copy(sink[0:64, 0:8], st["qw2"][0:64, 0:8])
            nc.vector.tensor_copy(sink[:, 8:16], att3[:, 0, 0:8])
            nc.vector.tensor_copy(sink[:, 16:24], wtm[:, 0, 0:8])
            nc.vector.tensor_copy(sink[:, 24:32], ctxT16[:, 0, 0:8])
            nc.sync.dma_start(dram["out"][b, 0:128, 0:32], sink[:])
            st["skip_final"] = True

    def _final(st, cs_range=range(8)):
        # ---- final matmul: out = (attended @ W^T + b) * mask ----
        if st.get("skip_final"):
            return
        b, ctxT16 = st["b"], st["ctxT16"]
        att3, wtm, mask_c = st["att3"], st["wtm"], st["mask_c"]
        pnt, qw2 = st["pnt"], st["qw2"]
        for cs in cs_range:
            csl = slice(cs * 128, (cs + 1) * 128)
            out_s = outp.tile([128, FE], BF16, tag="out")
            # lhsT-major over both fh psum tiles: consecutive matmuls share
            # the stationary operand, so the PE can skip/overlap reloads.
            # P-hat block (K=64), ctx*c2q (K=256), merged ctx block
            # (K=256; wtm last -- it is the latest-arriving operand)
            ps_o2 = [
                ps_fin.tile([128, 512], F32, tag="fin", name=f"fin{fh}")
                for fh in range(2)
            ]
            lhs_list = [
                (pnt[:, csl], [qw2[:, 0:512], qw2[:, 512:1024]]),
                (att3[:, 0, csl], [wt[:, 4, 0:512], wt[:, 4, 512:1024]]),
                (att3[:, 1, csl], [wt[:, 5, 0:512], wt[:, 5, 512:1024]]),
                (ctxT16[:, 0, csl], [wtm[:, 0, 0:512], wtm[:, 0, 512:1024]]),
                (ctxT16[:, 1, csl], [wtm[:, 1, 0:512], wtm[:, 1, 512:1024]]),
            ]
            for li, (lhsT, rhs2) in enumerate(lhs_list):
                for fh in range(2):
                    nc.tensor.matmul(
                        ps_o2[fh][:], lhsT, rhs2[fh],
                        start=(li == 0), stop=(li == 4),
                    )
            for fh in range(2):
                fhl = slice(fh * 512, (fh + 1) * 512)
                ps_o = ps_o2[fh]
                if mask_ones:
                    # evictions split ACT(fh0) / DVE(fh1): with the SBUF-only
                    # elementwise work on Pool, DVE's queue ahead of these is
                    # just reduces + att3, so the split stays balanced
                    if fh == 0:
                        nc.scalar.copy(out_s[:, fhl], ps_o[:])
                    else:
                        nc.vector.tensor_copy(out_s[:, fhl], ps_o[:])
                else:
                    nc.scalar.activation(
                        out=out_s[:, fhl],
                        in_=ps_o[:],
                        func=ACTF.Copy,
                        scale=mask_c[:, cs : cs + 1],
                    )
            nc.sync.dma_start(dram["out"][b, csl, :], out_s[:])

    def _step(prev, cur):
        # final(prev) interleaved with attn_B(cur): the softmax-dependent
        # PE ops of cur sit between final-tile groups so any residual wait
        # is absorbed mid-stream, and att3/wtm/pnt of cur are ready well
        # before final(cur) needs them.
        _final(prev, range(0, 3))
        _attn_B1(cur)
        _final(prev, range(3, 6))
        _attn_B2(cur)
        _final(prev, range(6, 8))

    def _all_batches(reps=1):
        # software pipeline: the final matmul of batch b runs between
        # attn_A(b+1) (sim matmuls + softmax issue) and around attn_B(b+1)
        # (PE ops that consume the softmax), so the in-order PE stream
        # always has final-matmul work queued while a softmax chain is in
        # flight. With reps>1 the whole flow is ONE continuous pipeline
        # over reps*BL batches -- no per-rep head/tail seams.
        seq = [b % BL for b in range(BL * reps)]
        sts = [_attn_A(seq[0])]
        if sts[0] is None:
            for i in range(1, len(seq)):
                _attn_A(seq[i])
            return
        prev = None
        for i in range(1, len(seq)):
            sts.append(
                _attn_A(seq[i], pair_st=sts[i - 1] if seq[i] % 2 == 1 else None)
            )
            if prev is None:
                _attn_B1(sts[i - 1])
                _attn_B2(sts[i - 1])
            else:
                _step(prev, sts[i - 1])
            prev = sts[i - 1]
        _step(prev, sts[-1])
        _final(sts[-1])

    if loop > 1:
        # Unroll several bodies per hardware-loop iteration: amortizes the
        # loop back-edge / cross-iteration refill cost per measured body.
        if unroll is None:
            unroll = 4
        while loop % unroll:
            unroll //= 2
        with tc.For_i(
            0,
            loop // unroll,
            1,
            staggered_reset=True,
            hint_engines=(
                mybir.EngineType.PE,
                mybir.EngineType.DVE,
                mybir.EngineType.Activation,
                mybir.EngineType.SP,
                mybir.EngineType.Pool,
            ),
        ):
            _all_batches(reps=unroll)
    else:
        _all_batches()
    if "stub" in dram:
        nc.sync.dma_start(dram["stub"][:], ones_f[0:1, 0:8])


_NC_CACHE = {}


def _get_nc(loop=1, bias_zero=False, mask_ones=False, probe=None, unroll=None):
    key = (loop, bias_zero, mask_ones, probe, unroll)
    if key not in _NC_CACHE:
        nc = bass.Bass("TRN2", target_bir_lowering=False, debug=False,
                       num_devices=NCORES)
        dram = {
            "ctxT16": nc.dram_tensor(
                "ctxT16", [BL, E, C], BF16, kind="ExternalInput"
            ).ap(),
            "ctxTh": nc.dram_tensor(
                "ctxTh", [BL, E, C], F16, kind="ExternalInput"
            ).ap(),
            "ctxn16": nc.dram_tensor(
                "ctxn16", [BL, C, E], BF16, kind="ExternalInput"
            ).ap(),
            "qT16": nc.dram_tensor("qT16", [BL, E, Q], BF16, kind="ExternalInput").ap(),
            "qTh": nc.dram_tensor("qTh", [BL, E, Q], F16, kind="ExternalInput").ap(),
            "q16": nc.dram_tensor("q16", [BL, Q, E], BF16, kind="ExternalInput").ap(),
            "mask": nc.dram_tensor("mask", [BL, C], F32, kind="ExternalInput").ap(),
            "wt16": nc.dram_tensor("wt16", [FE, FE], BF16, kind="ExternalInput").ap(),
            "b16": nc.dram_tensor("b16", [FE], BF16, kind="ExternalInput").ap(),
            "vecs": nc.dram_tensor("vecs", [E, 4], F32, kind="ExternalInput").ap(),
        }
        if loop > 1:
            # timing variant: keep the big output on-device, return a stub
            dram["out"] = nc.dram_tensor("out_int", [BL, C, FE], BF16).ap()
            dram["stub"] = nc.dram_tensor(
                "out", [1, 8], F32, kind="ExternalOutput"
            ).ap()
        else:
            dram["out"] = nc.dram_tensor(
                "out", [BL, C, FE], BF16, kind="ExternalOutput"
            ).ap()
        from contextlib import ExitStack

        with tile.TileContext(nc) as tc, ExitStack() as es:
            _emit(nc, tc, dram, es, loop=loop, bias_zero=bias_zero,
                  mask_ones=mask_ones, probe=probe, unroll=unroll)
        _split_multi_waits(nc)
        _NC_CACHE[key] = nc
    return _NC_CACHE[key]


def _prep_inputs(context, question, context_mask, w_question, w_context, w_multiple,
                 final_W, final_b):
    """Host-side layout prep + sharding. Returns per-core input maps."""
    bf16 = ml_dtypes.bfloat16
    context = np.asarray(context, np.float32)
    question = np.asarray(question, np.float32)
    ctxT = np.ascontiguousarray(context.transpose(0, 2, 1))
    ctxT16 = ctxT.astype(bf16)
    ctxTh = ctxT.astype(np.float16)
    ctx16 = context.astype(bf16)
    qT = np.ascontiguousarray(question.transpose(0, 2, 1))
    qT16 = qT.astype(bf16)
    qTh = qT.astype(np.float16)
    q16 = question.astype(bf16)
    wt16 = np.ascontiguousarray(np.asarray(final_W, np.float32).T).astype(bf16)
    b16 = np.asarray(final_b, np.float32).astype(bf16)
    vecs = np.stack(
        [
            np.asarray(w_question, np.float32),
            np.asarray(w_context, np.float32),
            np.asarray(w_multiple, np.float32),
            np.zeros(E, np.float32),
        ],
        axis=1,
    )
    mask = np.asarray(context_mask, np.float32)
    in_maps = []
    for i in range(NCORES):
        s = slice(i * BL, (i + 1) * BL)
        in_maps.append(
            {
                "ctxT16": ctxT16[s],
                "ctxTh": ctxTh[s],
                "ctxn16": ctx16[s],
                "qT16": qT16[s],
                "qTh": qTh[s],
                "q16": q16[s],
                "mask": mask[s],
                "wt16": wt16,
                "b16": b16,
                "vecs": vecs,
            }
        )
    return in_maps


def kernel(context, question, context_mask, w_question, w_context, w_multiple,
           final_W, final_b, _loop=1, _probe=None, _unroll=None, **run_kwargs):
    bias_zero = not np.any(np.asarray(final_b))
    mask_ones = bool(np.all(np.asarray(context_mask) == 1.0))
    nc = _get_nc(loop=_loop, bias_zero=bias_zero, mask_ones=mask_ones, probe=_probe,
                 unroll=_unroll)
    in_maps = _prep_inputs(
        context, question, context_mask, w_question, w_context, w_multiple,
        final_W, final_b,
    )
    res = run_bass_kernel_spmd(nc, in_maps, core_ids=list(range(NCORES)), **run_kwargs)
    if _loop > 1:
        return res
    out = np.empty((B, C, FE), np.float32)
    for i in range(NCORES):
        out[i * BL : (i + 1) * BL] = np.asarray(
            res.results[i]["out"], dtype=np.float32
        )
    if run_kwargs:
        kernel.last_results = res
    return out



# revision 51
# speedup vs baseline: 1.8829x; 1.8829x over previous
"""Trainium2 Bass kernel for BaseBidirectionalAttention (BiDAF-style attention).

Reference computation (per batch b):
    sim[c,q]  = <w_c, ctx_c> + <w_q, q_q> + <w_m, ctx_c * q_q>
    c2q       = softmax_q(sim) @ question                      (C, E)
    q2c_w     = softmax_c(max_q sim)                           (C,)
    q2c       = q2c_w @ context                                (E,)
    attended  = [ctx, c2q, ctx*c2q, ctx*q2c]                   (C, 4E)
    out       = (attended @ final_W.T + final_b) * mask[:,None] (C, 4E)

Sharding: data-parallel over batch. 32 batches / 8 cores = 4 per core.
All parameters (final_W etc., <5MB) replicated on every core.

Device-side layouts (prepared host-side; pure layout transforms):
    ctxT16 : context^T   (B, E, C)  bf16 (att3 source, final lhsT)
    ctxTh  : context^T   (B, E, C)  fp16 (sim lhsT: fp16 keeps softmax
             logits ~8x more exact than bf16 at the same PE rate)
    ctxn16 : context     (B, C, E)  bf16 (q2c contraction)
    qT16   : question^T  (B, E, Q)  bf16 (qw2 lhsT)
    qTh    : question^T  (B, E, Q)  fp16 (sim rhs source)
    q16    : question    bf16       (c2q lhsT)
    wt16   : final_W^T   (4E, 4E)   bf16 (final-matmul moving operand)

Key structure:
  - attended^T is built on-chip in bf16; the final matmul needs only
    K=576 of 1024: block4 (ctx*q2c) is folded into block1's weights
    (wtm = W1^T + q2c . W4^T) and block2 (c2q@W2^T = P @ (question@W2^T))
    contracts over Q=64 via qw2.
  - The final matmul runs lhsT-major over both 512-wide psum halves so
    each stationary operand is loaded once per two matmuls (the PE's
    double-buffered weights absorb the reload).
  - All matmuls are 16-bit (measured ~0.55-0.62 ns/psum-row on this HW vs
    2.5 ns/row fp32): sim logits in fp16 (bf16 logits flip near-tied
    softmax argmaxes and blow the max-norm error), the rest bf16.
  - q2c is contracted with ctx as 16 N=1 column matmuls (lhsT = ctx
    chunks) instead of 8 N=256 row matmuls -- PE cost is moving-row
    driven, so the column orientation is ~4x cheaper and lands q2c as
    the per-partition scalar wtm needs.
  - P^T via PE transposes, two 64-col chunks per 128x128 transpose.
  - Engine balance: PE does matmuls only; ACT does ONLY exps and half the
    final-matmul PSUM evictions; DVE does reduces + the other eviction
    half; Pool (gpsimd) does all SBUF-only elementwise work.
  - Softmax: one exp per 4-row-chunk half with a per-partition *block*
    max as bias (renormalization cancels exactly), p in bf16.
  - Schedule per batch: simMMs(b) | final(b-1) all tiles | softmax-
    dependent PE ops(b) -- PE never waits on the softmax chain.
  - Output is written bf16 (halves DMA); the host upcasts to fp32.
"""

import numpy as np
import ml_dtypes

import concourse.bass as bass
import concourse.mybir as mybir
import concourse.tile as tile
from concourse.bass_utils import run_bass_kernel_spmd
from concourse.masks import make_identity

B, C, Q, E = 32, 1024, 64, 256
FE = 4 * E
NCORES = 8
BL = B // NCORES  # batches per core

F32 = mybir.dt.float32
BF16 = mybir.dt.bfloat16
F16 = mybir.dt.float16
AX = mybir.AxisListType.X
AXY = mybir.AxisListType.XY
ALU = mybir.AluOpType
ACTF = mybir.ActivationFunctionType


def _split_multi_waits(nc):
    """The walrus build in this environment supports a single sync-wait per
    instruction. Move extra waits onto preceding same-engine NoOps."""
    counter = 0
    for f in nc.m.functions:
        for bb in f.blocks:
            insts = bb.instructions
            i = 0
            while i < len(insts):
                inst = insts[i]
                si = inst.sync_info
                waits = list(si.on_wait) if si is not None and si.on_wait else []
                if len(waits) > 1:
                    inst.sync_info = mybir.SyncInfo(
                        on_wait=[waits[-1]],
                        on_update=list(si.on_update) if si.on_update else [],
                    )
                    for w in waits[:-1]:
                        nop = mybir.InstNoOp(
                            name=f"I-swsplit-{counter}", engine=inst.engine
                        )
                        counter += 1
                        nop.sync_info = mybir.SyncInfo(on_wait=[w], on_update=[])
                        nc.register_instruction(nop)
                        insts.insert(i, nop)
                        i += 1
                i += 1


def _emit(nc, tc, dram, ctx, loop=1, bias_zero=False, mask_ones=False, probe=None,
          unroll=None):
    consts = ctx.enter_context(tc.tile_pool(name="consts", bufs=1))
    inp = ctx.enter_context(tc.tile_pool(name="inp", bufs=4))
    work = ctx.enter_context(tc.tile_pool(name="work", bufs=3))
    small = ctx.enter_context(tc.tile_pool(name="small", bufs=6))
    outp = ctx.enter_context(tc.tile_pool(name="outp", bufs=3))
    # PSUM: 8 banks total. Dedicated pools per phase so batch N+1's
    # attention never waits on batch N's final-matmul psums.
    ps_sim = ctx.enter_context(tc.tile_pool(name="ps_sim", bufs=2, space="PSUM"))
    ps_tp = ctx.enter_context(tc.tile_pool(name="ps_tp", bufs=1, space="PSUM"))
    ps_fin = ctx.enter_context(tc.tile_pool(name="ps_fin", bufs=3, space="PSUM"))
    ps_msc = ctx.enter_context(tc.tile_pool(name="ps_msc", bufs=2, space="PSUM"))

    # ---- constants ----
    wt = consts.tile([128, 8, FE], BF16)  # final_W^T, k-chunk major
    nc.sync.dma_start(wt[:], dram["wt16"].rearrange("(k p) f -> p k f", p=128))
    vecs = consts.tile([128, 2, 4], F32)  # cols: wq, wc, wm, 0 (e-chunked)
    nc.sync.dma_start(vecs[:], dram["vecs"].rearrange("(c p) v -> p c v", p=128))
    vecs16 = consts.tile([128, 2, 4], F16)
    nc.gpsimd.tensor_copy(vecs16[:], vecs[:])
    ones_f = consts.tile([1, 128], F32)
    nc.vector.memset(ones_f[:], 1.0)
    ones16 = consts.tile([1, 128], F16)
    nc.vector.memset(ones16[:], 1.0)
    ones_col = consts.tile([128, 1], F32)
    nc.vector.memset(ones_col[:], 1.0)
    ident = consts.tile([128, 128], F32)
    make_identity(nc, ident[:])
    ident16 = consts.tile([128, 128], BF16)
    nc.gpsimd.tensor_copy(ident16[:], ident[:])
    if not bias_zero:
        b_b128 = consts.tile([128, FE], BF16)
        nc.sync.dma_start(
            b_b128[:],
            bass.AP(
                tensor=dram["b16"].tensor,
                offset=dram["b16"].offset,
                ap=[[0, 128]] + list(dram["b16"].ap),
            ),
        )

    if probe == "dma":
        dma_src = [
            consts.tile([128, FE], BF16, tag=f"dmasrc{i}", name=f"dmasrc{i}")
            for i in range(2)
        ]
        for t in dma_src:
            nc.gpsimd.memset(t[:], 1.0)

    if probe == "pestream":
        pfake = consts.tile([128, 8, Q], BF16, tag="pfake")
        nc.gpsimd.memset(pfake[:], 0.01)
        e16fake = consts.tile([128, 8], BF16, tag="e16fake")
        nc.gpsimd.memset(e16fake[:], 0.01)

    if probe in ("peclock", "peclock2"):
        mmw = [
            consts.tile([128, 128], BF16, tag=f"mmw{i}", name=f"mmw{i}")
            for i in range(2)
        ]
        for t in mmw:
            nc.gpsimd.memset(t[:], 0.01)

    def _attn_A(b, pair_st=None):
        """Inputs + similarity matmuls + row softmax + qw2 (no PE ops that
        wait on the softmax chain)."""
        ctxT16 = inp.tile([128, 2, C], BF16, tag="ctxT16")
        nc.sync.dma_start(
            ctxT16[:], dram["ctxT16"][b].rearrange("(c p) n -> p c n", p=128)
        )
        ctxTh = inp.tile([128, 2, C], F16, tag="ctxTh")
        nc.sync.dma_start(
            ctxTh[:], dram["ctxTh"][b].rearrange("(c p) n -> p c n", p=128)
        )
        qTh = inp.tile([128, 2, Q], F16, tag="qTh")
        nc.sync.dma_start(
            qTh[:], dram["qTh"][b].rearrange("(c p) q -> p c q", p=128)
        )
        qT16 = inp.tile([128, 2, Q], BF16, tag="qT16")
        nc.sync.dma_start(
            qT16[:], dram["qT16"][b].rearrange("(c p) q -> p c q", p=128)
        )
        q16 = inp.tile([64, E], BF16, tag="q16")
        nc.sync.dma_start(q16[:], dram["q16"][b])
        ctxn16 = inp.tile([128, 8, E], BF16, tag="ctxn16")
        nc.sync.dma_start(
            ctxn16[:], dram["ctxn16"][b].rearrange("(j p) e -> p j e", p=128)
        )
        if not mask_ones:
            mask_row = inp.tile([1, C], F32, tag="mask")
            nc.sync.dma_start(mask_row[:], dram["mask"][b : b + 1, :])

        if probe == "dma":
            for cs in range(8):
                nc.sync.dma_start(
                    dram["out"][b, cs * 128 : (cs + 1) * 128, :], dma_src[cs % 2][:]
                )
            return None

        if probe in ("peclock", "peclock2"):
            # Pure PE streaming-rate probe: 96 N=512 bf16 matmuls per batch
            # over 4 psum banks (24-deep accumulation, one eviction per bank).
            # peclock: one stationary operand forever -> no reload cost.
            # peclock2: alternate two stationary operands -> reload per mm.
            for bank in range(4):
                ps_o = ps_fin.tile([128, 512], F32, tag="fin")
                for i in range(24):
                    lhsT = mmw[i % 2 if probe == "peclock2" else 0]
                    nc.tensor.matmul(
                        ps_o[:], lhsT[:], wt[:, i % 8, 0:512],
                        start=(i == 0), stop=(i == 23),
                    )
                s = small.tile([128, 1], F32, tag="mmsink")
                nc.vector.tensor_copy(s[:], ps_o[:, 0:1])
            return None

        if probe in ("mmonly", "mm256"):
            # dense self-loading bf16 matmuls, minimal eviction: HW ns/MM
            nf = 2 if probe == "mmonly" else 4
            fw = FE // nf
            for cs in range(8):
                csl = slice(cs * 128, (cs + 1) * 128)
                for fh in range(nf):
                    fhl = slice(fh * fw, (fh + 1) * fw)
                    ps_o = ps_fin.tile([128, fw], F32, tag="fin")
                    for kc in range(8):
                        nc.tensor.matmul(
                            ps_o[:], ctxT16[:, kc % 2, csl], wt[:, kc, fhl],
                            start=(kc == 0), stop=(kc == 7),
                        )
                    s = small.tile([128, 1], F32, tag="mmsink")
                    nc.vector.tensor_copy(s[:], ps_o[:, 0:1])
            return None

        if probe == "final":
            for cs in range(8):
                csl = slice(cs * 128, (cs + 1) * 128)
                out_s = outp.tile([128, FE], BF16, tag="out")
                for fh in range(2):
                    fhl = slice(fh * 512, (fh + 1) * 512)
                    ps_o = ps_fin.tile([128, 512], F32, tag="fin")
                    for kc in range(8):
                        nc.tensor.matmul(
                            ps_o[:], ctxT16[:, kc % 2, csl], wt[:, kc, fhl],
                            start=(kc == 0), stop=(kc == 7),
                        )
                    if fh == 0:
                        nc.scalar.copy(out_s[:, fhl], ps_o[:])
                    else:
                        nc.vector.tensor_copy(out_s[:, fhl], ps_o[:])
                nc.sync.dma_start(dram["out"][b, csl, :], out_s[:])
            return None

        if probe == "pestream":
            # The full per-batch PE stream with softmax/chain deps severed:
            # isolates how fast the PE mix itself can stream on HW.
            rhs_ext = work.tile([128, 2, Q + 1], F16, tag="qTs")
            for ec in range(2):
                nc.gpsimd.tensor_scalar_mul(
                    rhs_ext[:, ec, 0:Q], qTh[:, ec, :], vecs[:, ec, 2:3]
                )
                nc.gpsimd.tensor_copy(rhs_ext[:, ec, Q : Q + 1], vecs16[:, ec, 1:2])
            ps_qw = ps_msc.tile([1, Q], F32, tag="msc")
            for ec in range(2):
                nc.tensor.matmul(
                    ps_qw[:], vecs16[:, ec, 0:1], qTh[:, ec, :],
                    start=(ec == 0), stop=(ec == 1),
                )
            qsk = small.tile([1, 1], F32, tag="qwsink")
            nc.vector.tensor_copy(qsk[:], ps_qw[:, 0:1])
            for h in range(2):
                ps_s = ps_sim.tile([128, 4, Q + 1], F32, tag="sim")
                for k in range(4):
                    cs = h * 4 + k
                    csl = slice(cs * 128, (cs + 1) * 128)
                    nc.tensor.matmul(
                        ps_s[:, k, :], ctxTh[:, 0, csl], rhs_ext[:, 0, :],
                        start=(k == 0), stop=False,
                    )
                    nc.tensor.matmul(
                        ps_s[:, k, :], ctxTh[:, 1, csl], rhs_ext[:, 1, :],
                        start=False, stop=False,
                    )
                    nc.tensor.matmul(
                        ps_s[:, k, :], ones16[:], rhs_ext[0:1, 0, :],
                        start=False, stop=(k == 3),
                    )
                ssk = small.tile([128, 1], F32, tag="ssink")
                nc.vector.tensor_copy(ssk[:], ps_s[:, 0, 0:1])
            qw2 = work.tile([64, FE], BF16, tag="qw2")
            for fh in range(2):
                fhl = slice(fh * 512, (fh + 1) * 512)
                ps_q = ps_msc.tile([64, 512], F32, tag="msc")
                for ec in range(2):
                    nc.tensor.matmul(
                        ps_q[:], qT16[:, ec, :], wt[:, ec + 2, fhl],
                        start=(ec == 0), stop=(ec == 1),
                    )
                nc.scalar.copy(qw2[:, fhl], ps_q[:])
            pnt = work.tile([64, C], BF16, tag="pnt")
            for h in range(2):
                ps_t = ps_tp.tile([64, 4, 128], BF16, tag="tp")
                for k in range(4):
                    nc.tensor.transpose(
                        ps_t[:, k, :], pfake[:, h * 4 + k, :], ident16[:]
                    )
                nc.vector.tensor_copy(
                    pnt[:, h * 512 : (h + 1) * 512],
                    ps_t[:].rearrange("p a b -> p (a b)"),
                )
            att3 = work.tile([128, 2, C], BF16, tag="att3")
            for ec in range(2):
                for ch in range(2):
                    chl = slice(ch * 512, (ch + 1) * 512)
                    ps_c2q = ps_msc.tile([128, 512], F32, tag="msc")
                    nc.tensor.matmul(
                        ps_c2q[:], q16[:, ec * 128 : (ec + 1) * 128], pnt[:, chl],
                        start=True, stop=True,
                    )
                    nc.vector.tensor_mul(
                        att3[:, ec, chl], ctxT16[:, ec, chl], ps_c2q[:]
                    )
            for ec in range(2):
                ps_qc = ps_msc.tile([128, 1], F32, tag="msc")
                for j in range(8):
                    nc.tensor.matmul(
                        ps_qc[:],
                        ctxn16[:, j, ec * 128 : (ec + 1) * 128],
                        e16fake[:, j : j + 1],
                        start=(j == 0), stop=(j == 7),
                    )
                q2sk = small.tile([1, 1], F32, tag="q2sink")
                nc.vector.tensor_copy(q2sk[:], ps_qc[0:1, 0:1])
            for cs in range(8):
                csl = slice(cs * 128, (cs + 1) * 128)
                out_s = outp.tile([128, FE], BF16, tag="out")
                for fh in range(2):
                    fhl = slice(fh * 512, (fh + 1) * 512)
                    ps_o = ps_fin.tile([128, 512], F32, tag="fin")
                    nc.tensor.matmul(
                        ps_o[:], pnt[:, csl], qw2[:, fhl], start=True, stop=False
                    )
                    nc.tensor.matmul(
                        ps_o[:], att3[:, 0, csl], wt[:, 4, fhl],
                        start=False, stop=False,
                    )
                    nc.tensor.matmul(
                        ps_o[:], att3[:, 1, csl], wt[:, 5, fhl],
                        start=False, stop=False,
                    )
                    nc.tensor.matmul(
                        ps_o[:], ctxT16[:, 0, csl], wt[:, 0, fhl],
                        start=False, stop=False,
                    )
                    nc.tensor.matmul(
                        ps_o[:], ctxT16[:, 1, csl], wt[:, 1, fhl],
                        start=False, stop=True,
                    )
                    if fh == 0:
                        nc.scalar.copy(out_s[:, fhl], ps_o[:])
                    else:
                        nc.vector.tensor_copy(out_s[:, fhl], ps_o[:])
                nc.sync.dma_start(dram["out"][b, csl, :], out_s[:])
            return None

        # ---- rhs_ext = [qT * w_multiple | w_context]; col Q -> ctxw ----
        rhs_ext = work.tile([128, 2, Q + 1], F16, tag="qTs")
        for ec in range(2):
            nc.vector.tensor_scalar_mul(
                rhs_ext[:, ec, 0:Q], qTh[:, ec, :], vecs[:, ec, 2:3]
            )
            nc.vector.tensor_copy(rhs_ext[:, ec, Q : Q + 1], vecs16[:, ec, 1:2])

        # ---- q_weighted row: qw[q] = <w_question, question_q> ----
        ps_qw = ps_msc.tile([1, Q], F32, tag="msc")
        for ec in range(2):
            nc.tensor.matmul(
                ps_qw[:], vecs16[:, ec, 0:1], qTh[:, ec, :],
                start=(ec == 0), stop=(ec == 1),
            )
        # qw repeated 4x so ONE broadcast matmul covers a whole psum half
        qw_ext4 = work.tile([1, 4, Q + 1], F16, tag="qw")  # [qw | 0] x 4
        nc.vector.memset(qw_ext4[:], 0.0)
        for j in range(4):
            nc.vector.tensor_copy(qw_ext4[0:1, j, 0:Q], ps_qw[:])

        # ---- mask columns ----
        mask_c = None
        if not mask_ones:
            mask_c = work.tile([128, 8], F32, tag="mask_c")
            for cs in range(8):
                ps_mc = ps_msc.tile([128, 1], F32, tag="msc")
                nc.tensor.matmul(
                    ps_mc[:],
                    mask_row[0:1, cs * 128 : (cs + 1) * 128],
                    ones_f[0:1, 0:1],
                    start=True,
                    stop=True,
                )
                nc.vector.tensor_copy(mask_c[:, cs : cs + 1], ps_mc[:])

        # ---- similarity: logits (mult + qw[q]) cols 0..Q, ctxw[c] in col Q.
        # One exp per half with a per-partition BLOCK max as bias -- the
        # 1/Z normalization cancels the (rowmax - blockmax) offset exactly.
        negrow = work.tile([128, 8], F32, tag="negrow")
        bias_h = work.tile([128, 2], F32, tag="bias_h")
        ctxw_c = work.tile([128, 8], F32, tag="ctxw_c")
        p = work.tile([128, 8, Q], BF16, tag="p")
        zrows = work.tile([128, 8], F32, tag="zrows")
        rz = work.tile([128, 8], F32, tag="rz")
        for h in range(2):
            ps_s = ps_sim.tile([128, 4, Q + 1], F32, tag="sim")
            for k in range(4):
                cs = h * 4 + k
                csl = slice(cs * 128, (cs + 1) * 128)
                nc.tensor.matmul(
                    ps_s[:, k, :], ctxTh[:, 0, csl], rhs_ext[:, 0, :],
                    start=(k == 0), stop=False,
                )
                nc.tensor.matmul(
                    ps_s[:, k, :], ctxTh[:, 1, csl], rhs_ext[:, 1, :],
                    start=False, stop=False,
                )
            nc.tensor.matmul(
                ps_s[:, :, :], ones16[:], qw_ext4[0:1, :, :],
                start=False, stop=True,
            )
            hl = slice(h * 4, (h + 1) * 4)
            nc.vector.reduce_max(
                out=negrow[:, hl], in_=ps_s[:, :, 0:Q], axis=AX, negate=True
            )
            nc.vector.reduce_max(
                out=bias_h[:, h : h + 1], in_=ps_s[:, :, 0:Q], axis=AXY, negate=True
            )
            nc.vector.tensor_copy(
                ctxw_c[:, hl], ps_s[:, :, Q : Q + 1].rearrange("p a b -> p (a b)")
            )
            nc.scalar.activation(
                out=p[:, hl, :],
                in_=ps_s[:, :, 0:Q],
                func=ACTF.Exp,
                bias=bias_h[:, h : h + 1],
                scale=1.0,
            )
            nc.vector.reduce_sum(out=zrows[:, hl], in_=p[:, hl, :], axis=AX)
        # ---- q2c chain part A (serial small-op chain: kick it off as early
        # as possible so wtm construction in attn_B never gates the final
        # matmul). Produces e16 = exp(rowmax - gmax) and zrow2. ----
        rowtrue = work.tile([128, 8], F32, tag="rowtrue")
        nc.vector.tensor_sub(rowtrue[:], ctxw_c[:], negrow[:])
        # gmax in bf16 is fine: it is applied as the SAME bias to every row,
        # so any common inexactness cancels in the q2c softmax normalization.
        colmax = small.tile([128, 1], BF16, tag="colmax")
        nc.vector.reduce_max(out=colmax[:], in_=rowtrue[:], axis=AX)
        ps_t1 = ps_msc.tile([1, 128], BF16, tag="msc")
        nc.tensor.transpose(ps_t1[:], colmax[:], ident16[:])
        tmax = small.tile([1, 128], F32, tag="tmax")
        nc.vector.tensor_copy(tmax[:], ps_t1[:])
        gneg = small.tile([1, 1], F32, tag="gneg")  # -gmax
        nc.vector.reduce_max(out=gneg[:], in_=tmax[:], axis=AX, negate=True)
        ps_gb = ps_msc.tile([128, 1], F32, tag="msc")
        nc.tensor.matmul(ps_gb[:], ones_f[:], gneg[:], start=True, stop=True)
        gneg_col = small.tile([128, 1], F32, tag="gnegc")
        nc.vector.tensor_copy(gneg_col[:], ps_gb[:])
        e_t = work.tile([128, 8], F32, tag="e_t")  # exp(rowtrue - gmax)
        zrow2 = small.tile([128, 1], F32, tag="zrow2")
        nc.scalar.activation(
            out=e_t[:],
            in_=rowtrue[:],
            func=ACTF.Exp,
            bias=gneg_col[:],
            scale=1.0,
            accum_out=zrow2[:],
        )
        e16 = work.tile([128, 8], BF16, tag="e16")
        nc.vector.tensor_copy(e16[:], e_t[:])

        nc.vector.reciprocal(rz[:], zrows[:])
        for cs in range(8):
            nc.vector.tensor_scalar_mul(p[:, cs, :], p[:, cs, :], rz[:, cs : cs + 1])

        # ---- qW2 = question @ W2^T (+ bias: softmax rows sum to 1, so
        # adding b here adds exactly b to the output) ----
        qw2 = work.tile([64, FE], BF16, tag="qw2")
        for fh in range(2):
            fhl = slice(fh * 512, (fh + 1) * 512)
            ps_q = ps_msc.tile([64, 512], F32, tag="msc")
            for ec in range(2):
                nc.tensor.matmul(
                    ps_q[:], qT16[:, ec, :], wt[:, ec + 2, fhl],
                    start=(ec == 0), stop=(ec == 1),
                )
            if bias_zero:
                nc.scalar.copy(qw2[:, fhl], ps_q[:])
            else:
                nc.vector.tensor_add(qw2[:, fhl], ps_q[:], b_b128[0:64, fhl])

        return dict(
            b=b, ctxT16=ctxT16, q16=q16, ctxn16=ctxn16, mask_c=mask_c,
            p=p, qw2=qw2, e16=e16, zrow2=zrow2,
        )

    def _attn_B1(st):
        """P^T via PE transposes. Chunk PAIRS go through one [128,128]
        transpose (half the PE moving rows); the pair lands as q-rows 0:64
        (even chunk) / 64:128 (odd chunk) and is split at eviction."""
        p = st["p"]
        pnt = work.tile([64, C], BF16, tag="pnt")  # P_norm^T
        for h in range(2):
            ps_t = ps_tp.tile([128, 2, 128], BF16, tag="tp")
            for k in range(2):
                cs = h * 4 + 2 * k
                nc.tensor.transpose(
                    ps_t[:, k, :],
                    p[:, cs : cs + 2, :].rearrange("p a b -> p (a b)"),
                    ident16[:],
                )
            for k in range(2):
                cs = h * 4 + 2 * k
                nc.scalar.copy(
                    pnt[:, cs * 128 : (cs + 1) * 128], ps_t[0:64, k, :]
                )
                nc.scalar.copy(
                    pnt[:, (cs + 1) * 128 : (cs + 2) * 128], ps_t[64:128, k, :]
                )
        st["pnt"] = pnt

    def _attn_B2(st):
        """Remaining softmax-dependent ops: c2q, q2c, merged weights."""
        ctxT16, q16, ctxn16 = st["ctxT16"], st["q16"], st["ctxn16"]
        e16, zrow2, pnt = st["e16"], st["zrow2"], st["pnt"]

        # ---- c2q attention (only needed for block3 = ctx * c2q) ----
        att3 = work.tile([128, 2, C], BF16, tag="att3")  # (ctx*c2q)^T
        for ec in range(2):
            for ch in range(2):
                chl = slice(ch * 512, (ch + 1) * 512)
                ps_c2q = ps_msc.tile([128, 512], F32, tag="msc")
                nc.tensor.matmul(
                    ps_c2q[:],
                    q16[:, ec * 128 : (ec + 1) * 128],
                    pnt[:, chl],
                    start=True,
                    stop=True,
                )
                nc.vector.tensor_mul(att3[:, ec, chl], ctxT16[:, ec, chl], ps_c2q[:])

        # ---- q2c chain part B (e16/zrow2 were produced back in attn_A) ----
        ps_z = ps_msc.tile([1, 1], F32, tag="msc")
        nc.tensor.matmul(ps_z[:], zrow2[:], ones_col[:], start=True, stop=True)
        z_s = small.tile([1, 1], F32, tag="z_s")
        nc.vector.tensor_copy(z_s[:], ps_z[:])
        rz1 = small.tile([1, 1], F32, tag="rz1")
        nc.vector.reciprocal(rz1[:], z_s[:])
        # q2c computed TRANSPOSED: out [e, 1] columns directly (N=1 matmuls
        # are ~free: cost ~ per-instruction overhead, not 256 moving rows),
        # which also skips the row->column transpose matmuls for wtm.
        # block4 never materializes: (ctx . q2c) @ W4^T == ctx @ (q2c . W4^T),
        # so fold q2c into merged weights for the ctx block instead.
        wtm = work.tile([128, 2, FE], BF16, tag="wtm")  # W1^T + q2c . W4^T
        ps_rz = ps_msc.tile([128, 1], F32, tag="msc")
        nc.tensor.matmul(ps_rz[:], ones_f[:], rz1[:], start=True, stop=True)
        rz_col = small.tile([128, 1], F32, tag="rz_col")
        nc.vector.tensor_copy(rz_col[:], ps_rz[:])
        q2c_col2 = small.tile([128, 2], F32, tag="q2c_col2")
        for ec in range(2):
            ps_qc = ps_msc.tile([128, 1], F32, tag="msc")
            for j in range(8):
                nc.tensor.matmul(
                    ps_qc[:],
                    ctxn16[:, j, ec * 128 : (ec + 1) * 128],
                    e16[:, j : j + 1],
                    start=(j == 0),
                    stop=(j == 7),
                )
            # 1/Z folded into the eviction copy
            nc.vector.tensor_scalar_mul(q2c_col2[:, ec : ec + 1], ps_qc[:], rz_col[:])
        for ec in range(2):
            nc.vector.tensor_scalar_mul(
                wtm[:, ec, :], wt[:, ec + 6, :], q2c_col2[:, ec : ec + 1]
            )
            nc.vector.tensor_add(wtm[:, ec, :], wtm[:, ec, :], wt[:, ec, :])

        st["att3"], st["wtm"] = att3, wtm

        if probe == "attn":
            b = st["b"]
            sink = outp.tile([128, 32], BF16, tag="sink")
            nc.vector.tensor_copy(sink[0:64, 0:8], st["qw2"][0:64, 0:8])
            nc.vector.tensor_copy(sink[:, 8:16], att3[:, 0, 0:8])
            nc.vector.tensor_copy(sink[:, 16:24], wtm[:, 0, 0:8])
            nc.vector.tensor_copy(sink[:, 24:32], ctxT16[:, 0, 0:8])
            nc.sync.dma_start(dram["out"][b, 0:128, 0:32], sink[:])
            st["skip_final"] = True

    def _final(st, cs_range=range(8)):
        # ---- final matmul: out = (attended @ W^T + b) * mask ----
        if st.get("skip_final"):
            return
        b, ctxT16 = st["b"], st["ctxT16"]
        att3, wtm, mask_c = st["att3"], st["wtm"], st["mask_c"]
        pnt, qw2 = st["pnt"], st["qw2"]
        for cs in cs_range:
            csl = slice(cs * 128, (cs + 1) * 128)
            out_s = outp.tile([128, FE], BF16, tag="out")
            # lhsT-major over both fh psum tiles: consecutive matmuls share
            # the stationary operand, so the PE can skip/overlap reloads.
            # P-hat block (K=64), ctx*c2q (K=256), merged ctx block
            # (K=256; wtm last -- it is the latest-arriving operand)
            ps_o2 = [
                ps_fin.tile([128, 512], F32, tag="fin", name=f"fin{fh}")
                for fh in range(2)
            ]
            lhs_list = [
                (pnt[:, csl], [qw2[:, 0:512], qw2[:, 512:1024]]),
                (att3[:, 0, csl], [wt[:, 4, 0:512], wt[:, 4, 512:1024]]),
                (att3[:, 1, csl], [wt[:, 5, 0:512], wt[:, 5, 512:1024]]),
                (ctxT16[:, 0, csl], [wtm[:, 0, 0:512], wtm[:, 0, 512:1024]]),
                (ctxT16[:, 1, csl], [wtm[:, 1, 0:512], wtm[:, 1, 512:1024]]),
            ]
            for li, (lhsT, rhs2) in enumerate(lhs_list):
                for fh in range(2):
                    nc.tensor.matmul(
                        ps_o2[fh][:], lhsT, rhs2[fh],
                        start=(li == 0), stop=(li == 4),
                    )
            for fh in range(2):
                fhl = slice(fh * 512, (fh + 1) * 512)
                ps_o = ps_o2[fh]
                if mask_ones:
                    # evictions on ACT except mid-batch fh1 on DVE: the
                    # tail tiles' evicts must not queue behind attn_B's
                    # DVE work (pnt/att3/chain), which precedes them
                    if fh == 0 or cs >= 6:
                        nc.scalar.copy(out_s[:, fhl], ps_o[:])
                    else:
                        nc.vector.tensor_copy(out_s[:, fhl], ps_o[:])
                else:
                    nc.scalar.activation(
                        out=out_s[:, fhl],
                        in_=ps_o[:],
                        func=ACTF.Copy,
                        scale=mask_c[:, cs : cs + 1],
                    )
            nc.sync.dma_start(dram["out"][b, csl, :], out_s[:])

    def _step(prev, cur):
        # final(prev) interleaved with attn_B(cur): the softmax-dependent
        # PE ops of cur sit between final-tile groups so any residual wait
        # is absorbed mid-stream, and att3/wtm/pnt of cur are ready well
        # before final(cur) needs them.
        _final(prev, range(0, 3))
        _attn_B1(cur)
        _final(prev, range(3, 6))
        _attn_B2(cur)
        _final(prev, range(6, 8))

    def _all_batches(reps=1):
        # software pipeline: the final matmul of batch b runs between
        # attn_A(b+1) (sim matmuls + softmax issue) and around attn_B(b+1)
        # (PE ops that consume the softmax), so the in-order PE stream
        # always has final-matmul work queued while a softmax chain is in
        # flight. With reps>1 the whole flow is ONE continuous pipeline
        # over reps*BL batches -- no per-rep head/tail seams.
        seq = [b % BL for b in range(BL * reps)]
        sts = [_attn_A(seq[0])]
        if sts[0] is None:
            for i in range(1, len(seq)):
                _attn_A(seq[i])
            return
        prev = None
        for i in range(1, len(seq)):
            sts.append(
                _attn_A(seq[i], pair_st=sts[i - 1] if seq[i] % 2 == 1 else None)
            )
            if prev is None:
                _attn_B1(sts[i - 1])
                _attn_B2(sts[i - 1])
            else:
                _step(prev, sts[i - 1])
            prev = sts[i - 1]
        _step(prev, sts[-1])
        _final(sts[-1])

    if loop > 1:
        # Unroll several bodies per hardware-loop iteration: amortizes the
        # loop back-edge / cross-iteration refill cost per measured body.
        if unroll is None:
            unroll = 4
        while loop % unroll:
            unroll //= 2
        with tc.For_i(
            0,
            loop // unroll,
            1,
            staggered_reset=True,
            hint_engines=(
                mybir.EngineType.PE,
                mybir.EngineType.DVE,
                mybir.EngineType.Activation,
                mybir.EngineType.SP,
                mybir.EngineType.Pool,
            ),
        ):
            _all_batches(reps=unroll)
    else:
        _all_batches()
    if "stub" in dram:
        nc.sync.dma_start(dram["stub"][:], ones_f[0:1, 0:8])


_NC_CACHE = {}


def _get_nc(loop=1, bias_zero=False, mask_ones=False, probe=None, unroll=None):
    key = (loop, bias_zero, mask_ones, probe, unroll)
    if key not in _NC_CACHE:
        nc = bass.Bass("TRN2", target_bir_lowering=False, debug=False,
                       num_devices=NCORES)
        dram = {
            "ctxT16": nc.dram_tensor(
                "ctxT16", [BL, E, C], BF16, kind="ExternalInput"
            ).ap(),
            "ctxTh": nc.dram_tensor(
                "ctxTh", [BL, E, C], F16, kind="ExternalInput"
            ).ap(),
            "ctxn16": nc.dram_tensor(
                "ctxn16", [BL, C, E], BF16, kind="ExternalInput"
            ).ap(),
            "qT16": nc.dram_tensor("qT16", [BL, E, Q], BF16, kind="ExternalInput").ap(),
            "qTh": nc.dram_tensor("qTh", [BL, E, Q], F16, kind="ExternalInput").ap(),
            "q16": nc.dram_tensor("q16", [BL, Q, E], BF16, kind="ExternalInput").ap(),
            "mask": nc.dram_tensor("mask", [BL, C], F32, kind="ExternalInput").ap(),
            "wt16": nc.dram_tensor("wt16", [FE, FE], BF16, kind="ExternalInput").ap(),
            "b16": nc.dram_tensor("b16", [FE], BF16, kind="ExternalInput").ap(),
            "vecs": nc.dram_tensor("vecs", [E, 4], F32, kind="ExternalInput").ap(),
        }
        if loop > 1:
            # timing variant: keep the big output on-device, return a stub
            dram["out"] = nc.dram_tensor("out_int", [BL, C, FE], BF16).ap()
            dram["stub"] = nc.dram_tensor(
                "out", [1, 8], F32, kind="ExternalOutput"
            ).ap()
        else:
            dram["out"] = nc.dram_tensor(
                "out", [BL, C, FE], BF16, kind="ExternalOutput"
            ).ap()
        from contextlib import ExitStack

        with tile.TileContext(nc) as tc, ExitStack() as es:
            _emit(nc, tc, dram, es, loop=loop, bias_zero=bias_zero,
                  mask_ones=mask_ones, probe=probe, unroll=unroll)
        _split_multi_waits(nc)
        _NC_CACHE[key] = nc
    return _NC_CACHE[key]


def _prep_inputs(context, question, context_mask, w_question, w_context, w_multiple,
                 final_W, final_b):
    """Host-side layout prep + sharding. Returns per-core input maps."""
    bf16 = ml_dtypes.bfloat16
    context = np.asarray(context, np.float32)
    question = np.asarray(question, np.float32)
    ctxT = np.ascontiguousarray(context.transpose(0, 2, 1))
    ctxT16 = ctxT.astype(bf16)
    ctxTh = ctxT.astype(np.float16)
    ctx16 = context.astype(bf16)
    qT = np.ascontiguousarray(question.transpose(0, 2, 1))
    qT16 = qT.astype(bf16)
    qTh = qT.astype(np.float16)
    q16 = question.astype(bf16)
    wt16 = np.ascontiguousarray(np.asarray(final_W, np.float32).T).astype(bf16)
    b16 = np.asarray(final_b, np.float32).astype(bf16)
    vecs = np.stack(
        [
            np.asarray(w_question, np.float32),
            np.asarray(w_context, np.float32),
            np.asarray(w_multiple, np.float32),
            np.zeros(E, np.float32),
        ],
        axis=1,
    )
    mask = np.asarray(context_mask, np.float32)
    in_maps = []
    for i in range(NCORES):
        s = slice(i * BL, (i + 1) * BL)
        in_maps.append(
            {
                "ctxT16": ctxT16[s],
                "ctxTh": ctxTh[s],
                "ctxn16": ctx16[s],
                "qT16": qT16[s],
                "qTh": qTh[s],
                "q16": q16[s],
                "mask": mask[s],
                "wt16": wt16,
                "b16": b16,
                "vecs": vecs,
            }
        )
    return in_maps


def kernel(context, question, context_mask, w_question, w_context, w_multiple,
           final_W, final_b, _loop=1, _probe=None, _unroll=None, **run_kwargs):
    bias_zero = not np.any(np.asarray(final_b))
    mask_ones = bool(np.all(np.asarray(context_mask) == 1.0))
    nc = _get_nc(loop=_loop, bias_zero=bias_zero, mask_ones=mask_ones, probe=_probe,
                 unroll=_unroll)
    in_maps = _prep_inputs(
        context, question, context_mask, w_question, w_context, w_multiple,
        final_W, final_b,
    )
    res = run_bass_kernel_spmd(nc, in_maps, core_ids=list(range(NCORES)), **run_kwargs)
    if _loop > 1:
        return res
    out = np.empty((B, C, FE), np.float32)
    for i in range(NCORES):
        out[i * BL : (i + 1) * BL] = np.asarray(
            res.results[i]["out"], dtype=np.float32
        )
    if run_kwargs:
        kernel.last_results = res
    return out



# revision 52
# speedup vs baseline: 1.8839x; 1.0005x over previous
"""Trainium2 Bass kernel for BaseBidirectionalAttention (BiDAF-style attention).

Reference computation (per batch b):
    sim[c,q]  = <w_c, ctx_c> + <w_q, q_q> + <w_m, ctx_c * q_q>
    c2q       = softmax_q(sim) @ question                      (C, E)
    q2c_w     = softmax_c(max_q sim)                           (C,)
    q2c       = q2c_w @ context                                (E,)
    attended  = [ctx, c2q, ctx*c2q, ctx*q2c]                   (C, 4E)
    out       = (attended @ final_W.T + final_b) * mask[:,None] (C, 4E)

Sharding: data-parallel over batch. 32 batches / 8 cores = 4 per core.
All parameters (final_W etc., <5MB) replicated on every core.

Device-side layouts (prepared host-side; pure layout transforms):
    ctxT16 : context^T   (B, E, C)  bf16 (att3 source, final lhsT)
    ctxTh  : context^T   (B, E, C)  fp16 (sim lhsT: fp16 keeps softmax
             logits ~8x more exact than bf16 at the same PE rate)
    ctxn16 : context     (B, C, E)  bf16 (q2c contraction)
    qT16   : question^T  (B, E, Q)  bf16 (qw2 lhsT)
    qTh    : question^T  (B, E, Q)  fp16 (sim rhs source)
    q16    : question    bf16       (c2q lhsT)
    wt16   : final_W^T   (4E, 4E)   bf16 (final-matmul moving operand)

Key structure:
  - attended^T is built on-chip in bf16; the final matmul needs only
    K=576 of 1024: block4 (ctx*q2c) is folded into block1's weights
    (wtm = W1^T + q2c . W4^T) and block2 (c2q@W2^T = P @ (question@W2^T))
    contracts over Q=64 via qw2.
  - The final matmul runs lhsT-major over both 512-wide psum halves so
    each stationary operand is loaded once per two matmuls (the PE's
    double-buffered weights absorb the reload).
  - All matmuls are 16-bit (measured ~0.55-0.62 ns/psum-row on this HW vs
    2.5 ns/row fp32): sim logits in fp16 (bf16 logits flip near-tied
    softmax argmaxes and blow the max-norm error), the rest bf16.
  - q2c is contracted with ctx as 16 N=1 column matmuls (lhsT = ctx
    chunks) instead of 8 N=256 row matmuls -- PE cost is moving-row
    driven, so the column orientation is ~4x cheaper and lands q2c as
    the per-partition scalar wtm needs.
  - P^T via PE transposes, two 64-col chunks per 128x128 transpose.
  - Engine balance: PE does matmuls only; ACT does ONLY exps and half the
    final-matmul PSUM evictions; DVE does reduces + the other eviction
    half; Pool (gpsimd) does all SBUF-only elementwise work.
  - Softmax: one exp per 4-row-chunk half with a per-partition *block*
    max as bias (renormalization cancels exactly), p in bf16.
  - Schedule per batch: simMMs(b) | final(b-1) all tiles | softmax-
    dependent PE ops(b) -- PE never waits on the softmax chain.
  - Output is written bf16 (halves DMA); the host upcasts to fp32.
"""

import numpy as np
import ml_dtypes

import concourse.bass as bass
import concourse.mybir as mybir
import concourse.tile as tile
from concourse.bass_utils import run_bass_kernel_spmd
from concourse.masks import make_identity

B, C, Q, E = 32, 1024, 64, 256
FE = 4 * E
NCORES = 8
BL = B // NCORES  # batches per core

F32 = mybir.dt.float32
BF16 = mybir.dt.bfloat16
F16 = mybir.dt.float16
AX = mybir.AxisListType.X
AXY = mybir.AxisListType.XY
ALU = mybir.AluOpType
ACTF = mybir.ActivationFunctionType


def _split_multi_waits(nc):
    """The walrus build in this environment supports a single sync-wait per
    instruction. Move extra waits onto preceding same-engine NoOps."""
    counter = 0
    for f in nc.m.functions:
        for bb in f.blocks:
            insts = bb.instructions
            i = 0
            while i < len(insts):
                inst = insts[i]
                si = inst.sync_info
                waits = list(si.on_wait) if si is not None and si.on_wait else []
                if len(waits) > 1:
                    inst.sync_info = mybir.SyncInfo(
                        on_wait=[waits[-1]],
                        on_update=list(si.on_update) if si.on_update else [],
                    )
                    for w in waits[:-1]:
                        nop = mybir.InstNoOp(
                            name=f"I-swsplit-{counter}", engine=inst.engine
                        )
                        counter += 1
                        nop.sync_info = mybir.SyncInfo(on_wait=[w], on_update=[])
                        nc.register_instruction(nop)
                        insts.insert(i, nop)
                        i += 1
                i += 1


def _emit(nc, tc, dram, ctx, loop=1, bias_zero=False, mask_ones=False, probe=None,
          unroll=None):
    consts = ctx.enter_context(tc.tile_pool(name="consts", bufs=1))
    inp = ctx.enter_context(tc.tile_pool(name="inp", bufs=4))
    work = ctx.enter_context(tc.tile_pool(name="work", bufs=3))
    small = ctx.enter_context(tc.tile_pool(name="small", bufs=6))
    outp = ctx.enter_context(tc.tile_pool(name="outp", bufs=3))
    # PSUM: 8 banks total. Dedicated pools per phase so batch N+1's
    # attention never waits on batch N's final-matmul psums.
    ps_sim = ctx.enter_context(tc.tile_pool(name="ps_sim", bufs=1, space="PSUM"))
    ps_tp = ctx.enter_context(tc.tile_pool(name="ps_tp", bufs=1, space="PSUM"))
    ps_fin = ctx.enter_context(tc.tile_pool(name="ps_fin", bufs=4, space="PSUM"))
    ps_msc = ctx.enter_context(tc.tile_pool(name="ps_msc", bufs=2, space="PSUM"))

    # ---- constants ----
    wt = consts.tile([128, 8, FE], BF16)  # final_W^T, k-chunk major
    nc.sync.dma_start(wt[:], dram["wt16"].rearrange("(k p) f -> p k f", p=128))
    vecs = consts.tile([128, 2, 4], F32)  # cols: wq, wc, wm, 0 (e-chunked)
    nc.sync.dma_start(vecs[:], dram["vecs"].rearrange("(c p) v -> p c v", p=128))
    vecs16 = consts.tile([128, 2, 4], F16)
    nc.gpsimd.tensor_copy(vecs16[:], vecs[:])
    ones_f = consts.tile([1, 128], F32)
    nc.vector.memset(ones_f[:], 1.0)
    ones16 = consts.tile([1, 128], F16)
    nc.vector.memset(ones16[:], 1.0)
    ones_col = consts.tile([128, 1], F32)
    nc.vector.memset(ones_col[:], 1.0)
    ident = consts.tile([128, 128], F32)
    make_identity(nc, ident[:])
    ident16 = consts.tile([128, 128], BF16)
    nc.gpsimd.tensor_copy(ident16[:], ident[:])
    if not bias_zero:
        b_b128 = consts.tile([128, FE], BF16)
        nc.sync.dma_start(
            b_b128[:],
            bass.AP(
                tensor=dram["b16"].tensor,
                offset=dram["b16"].offset,
                ap=[[0, 128]] + list(dram["b16"].ap),
            ),
        )

    if probe == "dma":
        dma_src = [
            consts.tile([128, FE], BF16, tag=f"dmasrc{i}", name=f"dmasrc{i}")
            for i in range(2)
        ]
        for t in dma_src:
            nc.gpsimd.memset(t[:], 1.0)

    if probe == "pestream":
        pfake = consts.tile([128, 8, Q], BF16, tag="pfake")
        nc.gpsimd.memset(pfake[:], 0.01)
        e16fake = consts.tile([128, 8], BF16, tag="e16fake")
        nc.gpsimd.memset(e16fake[:], 0.01)

    if probe in ("peclock", "peclock2"):
        mmw = [
            consts.tile([128, 128], BF16, tag=f"mmw{i}", name=f"mmw{i}")
            for i in range(2)
        ]
        for t in mmw:
            nc.gpsimd.memset(t[:], 0.01)

    def _attn_A(b, pair_st=None):
        """Inputs + similarity matmuls + row softmax + qw2 (no PE ops that
        wait on the softmax chain)."""
        ctxT16 = inp.tile([128, 2, C], BF16, tag="ctxT16")
        nc.sync.dma_start(
            ctxT16[:], dram["ctxT16"][b].rearrange("(c p) n -> p c n", p=128)
        )
        ctxTh = inp.tile([128, 2, C], F16, tag="ctxTh")
        nc.sync.dma_start(
            ctxTh[:], dram["ctxTh"][b].rearrange("(c p) n -> p c n", p=128)
        )
        qTh = inp.tile([128, 2, Q], F16, tag="qTh")
        nc.sync.dma_start(
            qTh[:], dram["qTh"][b].rearrange("(c p) q -> p c q", p=128)
        )
        qT16 = inp.tile([128, 2, Q], BF16, tag="qT16")
        nc.sync.dma_start(
            qT16[:], dram["qT16"][b].rearrange("(c p) q -> p c q", p=128)
        )
        q16 = inp.tile([64, E], BF16, tag="q16")
        nc.sync.dma_start(q16[:], dram["q16"][b])
        ctxn16 = inp.tile([128, 8, E], BF16, tag="ctxn16")
        nc.sync.dma_start(
            ctxn16[:], dram["ctxn16"][b].rearrange("(j p) e -> p j e", p=128)
        )
        if not mask_ones:
            mask_row = inp.tile([1, C], F32, tag="mask")
            nc.sync.dma_start(mask_row[:], dram["mask"][b : b + 1, :])

        if probe == "dma":
            for cs in range(8):
                nc.sync.dma_start(
                    dram["out"][b, cs * 128 : (cs + 1) * 128, :], dma_src[cs % 2][:]
                )
            return None

        if probe in ("peclock", "peclock2"):
            # Pure PE streaming-rate probe: 96 N=512 bf16 matmuls per batch
            # over 4 psum banks (24-deep accumulation, one eviction per bank).
            # peclock: one stationary operand forever -> no reload cost.
            # peclock2: alternate two stationary operands -> reload per mm.
            for bank in range(4):
                ps_o = ps_fin.tile([128, 512], F32, tag="fin")
                for i in range(24):
                    lhsT = mmw[i % 2 if probe == "peclock2" else 0]
                    nc.tensor.matmul(
                        ps_o[:], lhsT[:], wt[:, i % 8, 0:512],
                        start=(i == 0), stop=(i == 23),
                    )
                s = small.tile([128, 1], F32, tag="mmsink")
                nc.vector.tensor_copy(s[:], ps_o[:, 0:1])
            return None

        if probe in ("mmonly", "mm256"):
            # dense self-loading bf16 matmuls, minimal eviction: HW ns/MM
            nf = 2 if probe == "mmonly" else 4
            fw = FE // nf
            for cs in range(8):
                csl = slice(cs * 128, (cs + 1) * 128)
                for fh in range(nf):
                    fhl = slice(fh * fw, (fh + 1) * fw)
                    ps_o = ps_fin.tile([128, fw], F32, tag="fin")
                    for kc in range(8):
                        nc.tensor.matmul(
                            ps_o[:], ctxT16[:, kc % 2, csl], wt[:, kc, fhl],
                            start=(kc == 0), stop=(kc == 7),
                        )
                    s = small.tile([128, 1], F32, tag="mmsink")
                    nc.vector.tensor_copy(s[:], ps_o[:, 0:1])
            return None

        if probe == "final":
            for cs in range(8):
                csl = slice(cs * 128, (cs + 1) * 128)
                out_s = outp.tile([128, FE], BF16, tag="out")
                for fh in range(2):
                    fhl = slice(fh * 512, (fh + 1) * 512)
                    ps_o = ps_fin.tile([128, 512], F32, tag="fin")
                    for kc in range(8):
                        nc.tensor.matmul(
                            ps_o[:], ctxT16[:, kc % 2, csl], wt[:, kc, fhl],
                            start=(kc == 0), stop=(kc == 7),
                        )
                    if fh == 0:
                        nc.scalar.copy(out_s[:, fhl], ps_o[:])
                    else:
                        nc.vector.tensor_copy(out_s[:, fhl], ps_o[:])
                nc.sync.dma_start(dram["out"][b, csl, :], out_s[:])
            return None

        if probe == "pestream":
            # The full per-batch PE stream with softmax/chain deps severed:
            # isolates how fast the PE mix itself can stream on HW.
            rhs_ext = work.tile([128, 2, Q + 1], F16, tag="qTs")
            for ec in range(2):
                nc.gpsimd.tensor_scalar_mul(
                    rhs_ext[:, ec, 0:Q], qTh[:, ec, :], vecs[:, ec, 2:3]
                )
                nc.gpsimd.tensor_copy(rhs_ext[:, ec, Q : Q + 1], vecs16[:, ec, 1:2])
            ps_qw = ps_msc.tile([1, Q], F32, tag="msc")
            for ec in range(2):
                nc.tensor.matmul(
                    ps_qw[:], vecs16[:, ec, 0:1], qTh[:, ec, :],
                    start=(ec == 0), stop=(ec == 1),
                )
            qsk = small.tile([1, 1], F32, tag="qwsink")
            nc.vector.tensor_copy(qsk[:], ps_qw[:, 0:1])
            for h in range(2):
                ps_s = ps_sim.tile([128, 4, Q + 1], F32, tag="sim")
                for k in range(4):
                    cs = h * 4 + k
                    csl = slice(cs * 128, (cs + 1) * 128)
                    nc.tensor.matmul(
                        ps_s[:, k, :], ctxTh[:, 0, csl], rhs_ext[:, 0, :],
                        start=(k == 0), stop=False,
                    )
                    nc.tensor.matmul(
                        ps_s[:, k, :], ctxTh[:, 1, csl], rhs_ext[:, 1, :],
                        start=False, stop=False,
                    )
                    nc.tensor.matmul(
                        ps_s[:, k, :], ones16[:], rhs_ext[0:1, 0, :],
                        start=False, stop=(k == 3),
                    )
                ssk = small.tile([128, 1], F32, tag="ssink")
                nc.vector.tensor_copy(ssk[:], ps_s[:, 0, 0:1])
            qw2 = work.tile([64, FE], BF16, tag="qw2")
            for fh in range(2):
                fhl = slice(fh * 512, (fh + 1) * 512)
                ps_q = ps_msc.tile([64, 512], F32, tag="msc")
                for ec in range(2):
                    nc.tensor.matmul(
                        ps_q[:], qT16[:, ec, :], wt[:, ec + 2, fhl],
                        start=(ec == 0), stop=(ec == 1),
                    )
                nc.scalar.copy(qw2[:, fhl], ps_q[:])
            pnt = work.tile([64, C], BF16, tag="pnt")
            for h in range(2):
                ps_t = ps_tp.tile([64, 4, 128], BF16, tag="tp")
                for k in range(4):
                    nc.tensor.transpose(
                        ps_t[:, k, :], pfake[:, h * 4 + k, :], ident16[:]
                    )
                nc.vector.tensor_copy(
                    pnt[:, h * 512 : (h + 1) * 512],
                    ps_t[:].rearrange("p a b -> p (a b)"),
                )
            att3 = work.tile([128, 2, C], BF16, tag="att3")
            for ec in range(2):
                for ch in range(2):
                    chl = slice(ch * 512, (ch + 1) * 512)
                    ps_c2q = ps_msc.tile([128, 512], F32, tag="msc")
                    nc.tensor.matmul(
                        ps_c2q[:], q16[:, ec * 128 : (ec + 1) * 128], pnt[:, chl],
                        start=True, stop=True,
                    )
                    nc.vector.tensor_mul(
                        att3[:, ec, chl], ctxT16[:, ec, chl], ps_c2q[:]
                    )
            for ec in range(2):
                ps_qc = ps_msc.tile([128, 1], F32, tag="msc")
                for j in range(8):
                    nc.tensor.matmul(
                        ps_qc[:],
                        ctxn16[:, j, ec * 128 : (ec + 1) * 128],
                        e16fake[:, j : j + 1],
                        start=(j == 0), stop=(j == 7),
                    )
                q2sk = small.tile([1, 1], F32, tag="q2sink")
                nc.vector.tensor_copy(q2sk[:], ps_qc[0:1, 0:1])
            for cs in range(8):
                csl = slice(cs * 128, (cs + 1) * 128)
                out_s = outp.tile([128, FE], BF16, tag="out")
                for fh in range(2):
                    fhl = slice(fh * 512, (fh + 1) * 512)
                    ps_o = ps_fin.tile([128, 512], F32, tag="fin")
                    nc.tensor.matmul(
                        ps_o[:], pnt[:, csl], qw2[:, fhl], start=True, stop=False
                    )
                    nc.tensor.matmul(
                        ps_o[:], att3[:, 0, csl], wt[:, 4, fhl],
                        start=False, stop=False,
                    )
                    nc.tensor.matmul(
                        ps_o[:], att3[:, 1, csl], wt[:, 5, fhl],
                        start=False, stop=False,
                    )
                    nc.tensor.matmul(
                        ps_o[:], ctxT16[:, 0, csl], wt[:, 0, fhl],
                        start=False, stop=False,
                    )
                    nc.tensor.matmul(
                        ps_o[:], ctxT16[:, 1, csl], wt[:, 1, fhl],
                        start=False, stop=True,
                    )
                    if fh == 0:
                        nc.scalar.copy(out_s[:, fhl], ps_o[:])
                    else:
                        nc.vector.tensor_copy(out_s[:, fhl], ps_o[:])
                nc.sync.dma_start(dram["out"][b, csl, :], out_s[:])
            return None

        # ---- rhs_ext = [qT * w_multiple | w_context]; col Q -> ctxw ----
        rhs_ext = work.tile([128, 2, Q + 1], F16, tag="qTs")
        for ec in range(2):
            nc.vector.tensor_scalar_mul(
                rhs_ext[:, ec, 0:Q], qTh[:, ec, :], vecs[:, ec, 2:3]
            )
            nc.vector.tensor_copy(rhs_ext[:, ec, Q : Q + 1], vecs16[:, ec, 1:2])

        # ---- q_weighted row: qw[q] = <w_question, question_q> ----
        ps_qw = ps_msc.tile([1, Q], F32, tag="msc")
        for ec in range(2):
            nc.tensor.matmul(
                ps_qw[:], vecs16[:, ec, 0:1], qTh[:, ec, :],
                start=(ec == 0), stop=(ec == 1),
            )
        # qw repeated 4x so ONE broadcast matmul covers a whole psum half
        qw_ext4 = work.tile([1, 4, Q + 1], F16, tag="qw")  # [qw | 0] x 4
        nc.vector.memset(qw_ext4[:], 0.0)
        for j in range(4):
            nc.vector.tensor_copy(qw_ext4[0:1, j, 0:Q], ps_qw[:])

        # ---- mask columns ----
        mask_c = None
        if not mask_ones:
            mask_c = work.tile([128, 8], F32, tag="mask_c")
            for cs in range(8):
                ps_mc = ps_msc.tile([128, 1], F32, tag="msc")
                nc.tensor.matmul(
                    ps_mc[:],
                    mask_row[0:1, cs * 128 : (cs + 1) * 128],
                    ones_f[0:1, 0:1],
                    start=True,
                    stop=True,
                )
                nc.vector.tensor_copy(mask_c[:, cs : cs + 1], ps_mc[:])

        # ---- similarity: logits (mult + qw[q]) cols 0..Q, ctxw[c] in col Q.
        # One exp per half with a per-partition BLOCK max as bias -- the
        # 1/Z normalization cancels the (rowmax - blockmax) offset exactly.
        negrow = work.tile([128, 8], F32, tag="negrow")
        bias_h = work.tile([128, 2], F32, tag="bias_h")
        ctxw_c = work.tile([128, 8], F32, tag="ctxw_c")
        p = work.tile([128, 8, Q], BF16, tag="p")
        zrows = work.tile([128, 8], F32, tag="zrows")
        rz = work.tile([128, 8], F32, tag="rz")
        for h in range(2):
            ps_s = ps_sim.tile([128, 4, Q + 1], F32, tag="sim")
            for k in range(4):
                cs = h * 4 + k
                csl = slice(cs * 128, (cs + 1) * 128)
                nc.tensor.matmul(
                    ps_s[:, k, :], ctxTh[:, 0, csl], rhs_ext[:, 0, :],
                    start=(k == 0), stop=False,
                )
                nc.tensor.matmul(
                    ps_s[:, k, :], ctxTh[:, 1, csl], rhs_ext[:, 1, :],
                    start=False, stop=False,
                )
            nc.tensor.matmul(
                ps_s[:, :, :], ones16[:], qw_ext4[0:1, :, :],
                start=False, stop=True,
            )
            hl = slice(h * 4, (h + 1) * 4)
            nc.vector.reduce_max(
                out=negrow[:, hl], in_=ps_s[:, :, 0:Q], axis=AX, negate=True
            )
            nc.vector.reduce_max(
                out=bias_h[:, h : h + 1], in_=ps_s[:, :, 0:Q], axis=AXY, negate=True
            )
            nc.vector.tensor_copy(
                ctxw_c[:, hl], ps_s[:, :, Q : Q + 1].rearrange("p a b -> p (a b)")
            )
            nc.scalar.activation(
                out=p[:, hl, :],
                in_=ps_s[:, :, 0:Q],
                func=ACTF.Exp,
                bias=bias_h[:, h : h + 1],
                scale=1.0,
            )
            nc.vector.reduce_sum(out=zrows[:, hl], in_=p[:, hl, :], axis=AX)
        # ---- q2c chain part A (serial small-op chain: kick it off as early
        # as possible so wtm construction in attn_B never gates the final
        # matmul). Produces e16 = exp(rowmax - gmax) and zrow2. ----
        rowtrue = work.tile([128, 8], F32, tag="rowtrue")
        nc.vector.tensor_sub(rowtrue[:], ctxw_c[:], negrow[:])
        # gmax in bf16 is fine: it is applied as the SAME bias to every row,
        # so any common inexactness cancels in the q2c softmax normalization.
        colmax = small.tile([128, 1], BF16, tag="colmax")
        nc.vector.reduce_max(out=colmax[:], in_=rowtrue[:], axis=AX)
        ps_t1 = ps_msc.tile([1, 128], BF16, tag="msc")
        nc.tensor.transpose(ps_t1[:], colmax[:], ident16[:])
        tmax = small.tile([1, 128], F32, tag="tmax")
        nc.vector.tensor_copy(tmax[:], ps_t1[:])
        gneg = small.tile([1, 1], F32, tag="gneg")  # -gmax
        nc.vector.reduce_max(out=gneg[:], in_=tmax[:], axis=AX, negate=True)
        ps_gb = ps_msc.tile([128, 1], F32, tag="msc")
        nc.tensor.matmul(ps_gb[:], ones_f[:], gneg[:], start=True, stop=True)
        gneg_col = small.tile([128, 1], F32, tag="gnegc")
        nc.vector.tensor_copy(gneg_col[:], ps_gb[:])
        e_t = work.tile([128, 8], F32, tag="e_t")  # exp(rowtrue - gmax)
        zrow2 = small.tile([128, 1], F32, tag="zrow2")
        nc.scalar.activation(
            out=e_t[:],
            in_=rowtrue[:],
            func=ACTF.Exp,
            bias=gneg_col[:],
            scale=1.0,
            accum_out=zrow2[:],
        )
        e16 = work.tile([128, 8], BF16, tag="e16")
        nc.vector.tensor_copy(e16[:], e_t[:])

        nc.vector.reciprocal(rz[:], zrows[:])
        for cs in range(8):
            nc.vector.tensor_scalar_mul(p[:, cs, :], p[:, cs, :], rz[:, cs : cs + 1])

        # ---- qW2 = question @ W2^T (+ bias: softmax rows sum to 1, so
        # adding b here adds exactly b to the output) ----
        qw2 = work.tile([64, FE], BF16, tag="qw2")
        for fh in range(2):
            fhl = slice(fh * 512, (fh + 1) * 512)
            ps_q = ps_msc.tile([64, 512], F32, tag="msc")
            for ec in range(2):
                nc.tensor.matmul(
                    ps_q[:], qT16[:, ec, :], wt[:, ec + 2, fhl],
                    start=(ec == 0), stop=(ec == 1),
                )
            if bias_zero:
                nc.scalar.copy(qw2[:, fhl], ps_q[:])
            else:
                nc.vector.tensor_add(qw2[:, fhl], ps_q[:], b_b128[0:64, fhl])

        return dict(
            b=b, ctxT16=ctxT16, q16=q16, ctxn16=ctxn16, mask_c=mask_c,
            p=p, qw2=qw2, e16=e16, zrow2=zrow2,
        )

    def _attn_B1(st):
        """P^T via PE transposes. Chunk PAIRS go through one [128,128]
        transpose (half the PE moving rows); the pair lands as q-rows 0:64
        (even chunk) / 64:128 (odd chunk) and is split at eviction."""
        p = st["p"]
        pnt = work.tile([64, C], BF16, tag="pnt")  # P_norm^T
        for h in range(2):
            ps_t = ps_tp.tile([128, 2, 128], BF16, tag="tp")
            for k in range(2):
                cs = h * 4 + 2 * k
                nc.tensor.transpose(
                    ps_t[:, k, :],
                    p[:, cs : cs + 2, :].rearrange("p a b -> p (a b)"),
                    ident16[:],
                )
            for k in range(2):
                cs = h * 4 + 2 * k
                nc.scalar.copy(
                    pnt[:, cs * 128 : (cs + 1) * 128], ps_t[0:64, k, :]
                )
                nc.scalar.copy(
                    pnt[:, (cs + 1) * 128 : (cs + 2) * 128], ps_t[64:128, k, :]
                )
        st["pnt"] = pnt

    def _attn_B2(st):
        """Remaining softmax-dependent ops: c2q, q2c, merged weights."""
        ctxT16, q16, ctxn16 = st["ctxT16"], st["q16"], st["ctxn16"]
        e16, zrow2, pnt = st["e16"], st["zrow2"], st["pnt"]

        # ---- c2q attention (only needed for block3 = ctx * c2q) ----
        att3 = work.tile([128, 2, C], BF16, tag="att3")  # (ctx*c2q)^T
        for ec in range(2):
            for ch in range(2):
                chl = slice(ch * 512, (ch + 1) * 512)
                ps_c2q = ps_msc.tile([128, 512], F32, tag="msc")
                nc.tensor.matmul(
                    ps_c2q[:],
                    q16[:, ec * 128 : (ec + 1) * 128],
                    pnt[:, chl],
                    start=True,
                    stop=True,
                )
                nc.vector.tensor_mul(att3[:, ec, chl], ctxT16[:, ec, chl], ps_c2q[:])

        # ---- q2c chain part B (e16/zrow2 were produced back in attn_A) ----
        ps_z = ps_msc.tile([1, 1], F32, tag="msc")
        nc.tensor.matmul(ps_z[:], zrow2[:], ones_col[:], start=True, stop=True)
        z_s = small.tile([1, 1], F32, tag="z_s")
        nc.vector.tensor_copy(z_s[:], ps_z[:])
        rz1 = small.tile([1, 1], F32, tag="rz1")
        nc.vector.reciprocal(rz1[:], z_s[:])
        # q2c computed TRANSPOSED: out [e, 1] columns directly (N=1 matmuls
        # are ~free: cost ~ per-instruction overhead, not 256 moving rows),
        # which also skips the row->column transpose matmuls for wtm.
        # block4 never materializes: (ctx . q2c) @ W4^T == ctx @ (q2c . W4^T),
        # so fold q2c into merged weights for the ctx block instead.
        wtm = work.tile([128, 2, FE], BF16, tag="wtm")  # W1^T + q2c . W4^T
        ps_rz = ps_msc.tile([128, 1], F32, tag="msc")
        nc.tensor.matmul(ps_rz[:], ones_f[:], rz1[:], start=True, stop=True)
        rz_col = small.tile([128, 1], F32, tag="rz_col")
        nc.vector.tensor_copy(rz_col[:], ps_rz[:])
        q2c_col2 = small.tile([128, 2], F32, tag="q2c_col2")
        for ec in range(2):
            ps_qc = ps_msc.tile([128, 1], F32, tag="msc")
            for j in range(8):
                nc.tensor.matmul(
                    ps_qc[:],
                    ctxn16[:, j, ec * 128 : (ec + 1) * 128],
                    e16[:, j : j + 1],
                    start=(j == 0),
                    stop=(j == 7),
                )
            # 1/Z folded into the eviction copy
            nc.vector.tensor_scalar_mul(q2c_col2[:, ec : ec + 1], ps_qc[:], rz_col[:])
        for ec in range(2):
            nc.vector.tensor_scalar_mul(
                wtm[:, ec, :], wt[:, ec + 6, :], q2c_col2[:, ec : ec + 1]
            )
            nc.vector.tensor_add(wtm[:, ec, :], wtm[:, ec, :], wt[:, ec, :])

        st["att3"], st["wtm"] = att3, wtm

        if probe == "attn":
            b = st["b"]
            sink = outp.tile([128, 32], BF16, tag="sink")
            nc.vector.tensor_copy(sink[0:64, 0:8], st["qw2"][0:64, 0:8])
            nc.vector.tensor_copy(sink[:, 8:16], att3[:, 0, 0:8])
            nc.vector.tensor_copy(sink[:, 16:24], wtm[:, 0, 0:8])
            nc.vector.tensor_copy(sink[:, 24:32], ctxT16[:, 0, 0:8])
            nc.sync.dma_start(dram["out"][b, 0:128, 0:32], sink[:])
            st["skip_final"] = True

    def _final(st, cs_range=range(8)):
        # ---- final matmul: out = (attended @ W^T + b) * mask ----
        if st.get("skip_final"):
            return
        b, ctxT16 = st["b"], st["ctxT16"]
        att3, wtm, mask_c = st["att3"], st["wtm"], st["mask_c"]
        pnt, qw2 = st["pnt"], st["qw2"]
        for cs in cs_range:
            csl = slice(cs * 128, (cs + 1) * 128)
            out_s = outp.tile([128, FE], BF16, tag="out")
            # lhsT-major over both fh psum tiles: consecutive matmuls share
            # the stationary operand, so the PE can skip/overlap reloads.
            # P-hat block (K=64), ctx*c2q (K=256), merged ctx block
            # (K=256; wtm last -- it is the latest-arriving operand)
            ps_o2 = [
                ps_fin.tile([128, 512], F32, tag="fin", name=f"fin{fh}")
                for fh in range(2)
            ]
            lhs_list = [
                (pnt[:, csl], [qw2[:, 0:512], qw2[:, 512:1024]]),
                (att3[:, 0, csl], [wt[:, 4, 0:512], wt[:, 4, 512:1024]]),
                (att3[:, 1, csl], [wt[:, 5, 0:512], wt[:, 5, 512:1024]]),
                (ctxT16[:, 0, csl], [wtm[:, 0, 0:512], wtm[:, 0, 512:1024]]),
                (ctxT16[:, 1, csl], [wtm[:, 1, 0:512], wtm[:, 1, 512:1024]]),
            ]
            for li, (lhsT, rhs2) in enumerate(lhs_list):
                for fh in range(2):
                    nc.tensor.matmul(
                        ps_o2[fh][:], lhsT, rhs2[fh],
                        start=(li == 0), stop=(li == 4),
                    )
            for fh in range(2):
                fhl = slice(fh * 512, (fh + 1) * 512)
                ps_o = ps_o2[fh]
                if mask_ones:
                    # evictions on ACT except mid-batch fh1 on DVE: the
                    # tail tiles' evicts must not queue behind attn_B's
                    # DVE work (pnt/att3/chain), which precedes them
                    if fh == 0 or cs >= 6:
                        nc.scalar.copy(out_s[:, fhl], ps_o[:])
                    else:
                        nc.vector.tensor_copy(out_s[:, fhl], ps_o[:])
                else:
                    nc.scalar.activation(
                        out=out_s[:, fhl],
                        in_=ps_o[:],
                        func=ACTF.Copy,
                        scale=mask_c[:, cs : cs + 1],
                    )
            nc.sync.dma_start(dram["out"][b, csl, :], out_s[:])

    def _step(prev, cur):
        # final(prev) interleaved with attn_B(cur): the softmax-dependent
        # PE ops of cur sit between final-tile groups so any residual wait
        # is absorbed mid-stream, and att3/wtm/pnt of cur are ready well
        # before final(cur) needs them.
        _final(prev, range(0, 3))
        _attn_B1(cur)
        _final(prev, range(3, 6))
        _attn_B2(cur)
        _final(prev, range(6, 8))

    def _all_batches(reps=1):
        # software pipeline: the final matmul of batch b runs between
        # attn_A(b+1) (sim matmuls + softmax issue) and around attn_B(b+1)
        # (PE ops that consume the softmax), so the in-order PE stream
        # always has final-matmul work queued while a softmax chain is in
        # flight. With reps>1 the whole flow is ONE continuous pipeline
        # over reps*BL batches -- no per-rep head/tail seams.
        seq = [b % BL for b in range(BL * reps)]
        sts = [_attn_A(seq[0])]
        if sts[0] is None:
            for i in range(1, len(seq)):
                _attn_A(seq[i])
            return
        prev = None
        for i in range(1, len(seq)):
            sts.append(
                _attn_A(seq[i], pair_st=sts[i - 1] if seq[i] % 2 == 1 else None)
            )
            if prev is None:
                _attn_B1(sts[i - 1])
                _attn_B2(sts[i - 1])
            else:
                _step(prev, sts[i - 1])
            prev = sts[i - 1]
        _step(prev, sts[-1])
        _final(sts[-1])

    if loop > 1:
        # Unroll several bodies per hardware-loop iteration: amortizes the
        # loop back-edge / cross-iteration refill cost per measured body.
        if unroll is None:
            unroll = 4
        while loop % unroll:
            unroll //= 2
        with tc.For_i(
            0,
            loop // unroll,
            1,
            staggered_reset=True,
            hint_engines=(
                mybir.EngineType.PE,
                mybir.EngineType.DVE,
                mybir.EngineType.Activation,
                mybir.EngineType.SP,
                mybir.EngineType.Pool,
            ),
        ):
            _all_batches(reps=unroll)
    else:
        _all_batches()
    if "stub" in dram:
        nc.sync.dma_start(dram["stub"][:], ones_f[0:1, 0:8])


_NC_CACHE = {}


def _get_nc(loop=1, bias_zero=False, mask_ones=False, probe=None, unroll=None):
    key = (loop, bias_zero, mask_ones, probe, unroll)
    if key not in _NC_CACHE:
        nc = bass.Bass("TRN2", target_bir_lowering=False, debug=False,
                       num_devices=NCORES)
        dram = {
            "ctxT16": nc.dram_tensor(
                "ctxT16", [BL, E, C], BF16, kind="ExternalInput"
            ).ap(),
            "ctxTh": nc.dram_tensor(
                "ctxTh", [BL, E, C], F16, kind="ExternalInput"
            ).ap(),
            "ctxn16": nc.dram_tensor(
                "ctxn16", [BL, C, E], BF16, kind="ExternalInput"
            ).ap(),
            "qT16": nc.dram_tensor("qT16", [BL, E, Q], BF16, kind="ExternalInput").ap(),
            "qTh": nc.dram_tensor("qTh", [BL, E, Q], F16, kind="ExternalInput").ap(),
            "q16": nc.dram_tensor("q16", [BL, Q, E], BF16, kind="ExternalInput").ap(),
            "mask": nc.dram_tensor("mask", [BL, C], F32, kind="ExternalInput").ap(),
            "wt16": nc.dram_tensor("wt16", [FE, FE], BF16, kind="ExternalInput").ap(),
            "b16": nc.dram_tensor("b16", [FE], BF16, kind="ExternalInput").ap(),
            "vecs": nc.dram_tensor("vecs", [E, 4], F32, kind="ExternalInput").ap(),
        }
        if loop > 1:
            # timing variant: keep the big output on-device, return a stub
            dram["out"] = nc.dram_tensor("out_int", [BL, C, FE], BF16).ap()
            dram["stub"] = nc.dram_tensor(
                "out", [1, 8], F32, kind="ExternalOutput"
            ).ap()
        else:
            dram["out"] = nc.dram_tensor(
                "out", [BL, C, FE], BF16, kind="ExternalOutput"
            ).ap()
        from contextlib import ExitStack

        with tile.TileContext(nc) as tc, ExitStack() as es:
            _emit(nc, tc, dram, es, loop=loop, bias_zero=bias_zero,
                  mask_ones=mask_ones, probe=probe, unroll=unroll)
        _split_multi_waits(nc)
        _NC_CACHE[key] = nc
    return _NC_CACHE[key]


def _prep_inputs(context, question, context_mask, w_question, w_context, w_multiple,
                 final_W, final_b):
    """Host-side layout prep + sharding. Returns per-core input maps."""
    bf16 = ml_dtypes.bfloat16
    context = np.asarray(context, np.float32)
    question = np.asarray(question, np.float32)
    ctxT = np.ascontiguousarray(context.transpose(0, 2, 1))
    ctxT16 = ctxT.astype(bf16)
    ctxTh = ctxT.astype(np.float16)
    ctx16 = context.astype(bf16)
    qT = np.ascontiguousarray(question.transpose(0, 2, 1))
    qT16 = qT.astype(bf16)
    qTh = qT.astype(np.float16)
    q16 = question.astype(bf16)
    wt16 = np.ascontiguousarray(np.asarray(final_W, np.float32).T).astype(bf16)
    b16 = np.asarray(final_b, np.float32).astype(bf16)
    vecs = np.stack(
        [
            np.asarray(w_question, np.float32),
            np.asarray(w_context, np.float32),
            np.asarray(w_multiple, np.float32),
            np.zeros(E, np.float32),
        ],
        axis=1,
    )
    mask = np.asarray(context_mask, np.float32)
    in_maps = []
    for i in range(NCORES):
        s = slice(i * BL, (i + 1) * BL)
        in_maps.append(
            {
                "ctxT16": ctxT16[s],
                "ctxTh": ctxTh[s],
                "ctxn16": ctx16[s],
                "qT16": qT16[s],
                "qTh": qTh[s],
                "q16": q16[s],
                "mask": mask[s],
                "wt16": wt16,
                "b16": b16,
                "vecs": vecs,
            }
        )
    return in_maps


def kernel(context, question, context_mask, w_question, w_context, w_multiple,
           final_W, final_b, _loop=1, _probe=None, _unroll=None, **run_kwargs):
    bias_zero = not np.any(np.asarray(final_b))
    mask_ones = bool(np.all(np.asarray(context_mask) == 1.0))
    nc = _get_nc(loop=_loop, bias_zero=bias_zero, mask_ones=mask_ones, probe=_probe,
                 unroll=_unroll)
    in_maps = _prep_inputs(
        context, question, context_mask, w_question, w_context, w_multiple,
        final_W, final_b,
    )
    res = run_bass_kernel_spmd(nc, in_maps, core_ids=list(range(NCORES)), **run_kwargs)
    if _loop > 1:
        return res
    out = np.empty((B, C, FE), np.float32)
    for i in range(NCORES):
        out[i * BL : (i + 1) * BL] = np.asarray(
            res.results[i]["out"], dtype=np.float32
        )
    if run_kwargs:
        kernel.last_results = res
    return out

